# revision 2
# baseline (speedup 1.0000x reference)
"""Multi-head attention on 8 Trainium2 NeuronCores — fp8-DoubleRow edition.

Problem: x[4, 2048, 768] -> qkv (12 heads, d=64) -> softmax attention -> proj.

Sharding: data-parallel over batch (4) x tensor-parallel over heads (2
groups of 6 heads) -> 8 shards; the host adds the two partial projections
per batch plus the (qkv v-bias + proj bias) fold (pure numpy adds).

All heavy matmuls run as fp8e4m3 MatmulPerfMode.DoubleRow (0.5 cycles/row;
pairing two 128-deep contraction tiles per matmul halves the row count
again), taking per-core PE busy from ~228us (f32r baseline) to ~133us. The
bottleneck becomes the ACT/DVE elementwise wall: softmax exp of 25.2M
logits/core plus all PSUM evacuation (Pool/GPSIMD cannot access PSUM and
DMA cannot read PSUM, so exactly two engines can touch PSUM). The exp is
split ACT (exact table exp -> fp8, ln16 bias) / DVE (Schraudolph integer
exp: e4m3 bits = int8(A8*x + B8), ~3% sawtooth) roughly 34:30 per pair;
Pool handles the softmax normalization (broadcast + multiply, all SBUF).

Numerics (e4m3 denormals start at 2^-6, so every small tensor is quantized
at a power-of-2 scale, undone downstream for free): weights at 32x (undone
in the evac affine), q/k at 4x (undone in the exp argument), v at 16x with
the PV sums column holding 16.0 so normalization cancels everything.
Accuracy is bought with residual terms where it is cheap: q/k = (x8 + xr) @
w8 + x8 @ wr (host-prepared fp8 residual tensors), and V feeds PV as TWO
fp8 tensors v_hi + v_lo with two DoubleRow accumulation chains (~bf16
quality at fp8 speed). pt is fp8 at 16x. proj stays f32r (fp8 would cost
2.4e-2 rel err alone; 3-term fp8 proj is exactly cost-neutral vs f32r with
an odd k-tile count). Output DMA is fp16, upcast host-side.

Hardware gotchas found on the way: dual-fp8 LdWeights requires the pair
stride to be a multiple of 128B (V is stored in 128B-aligned per-head
slots); a PSUM start=True zeroes the WHOLE 2KB bank, so exactly one matmul
per bank carries start; the QK DoubleRow puts the 64-channel contraction in
pair0 and zeros in pair1 so no partition remap of q/k is ever needed.

Schedule: sweep1 (PE-bound, ~44us) computes q/k for all pairs + V; three
fillerless attention pairs (~45us each, exp-bound, 3 st psum bufs so QK
never sits on the exp critical path; the normalization chain is deferred 4
tiles so the DVE reciprocal never heads the queue); f32r projection tail.
Cost-model span 205.5us (baseline 254.1us); measured rel err 1.64e-2
(gate 2e-2).
"""

import sys

sys.path.insert(0, "/opt/trn_rl_repo")

import numpy as np
import ml_dtypes

import concourse.bass as bass
import concourse.mybir as mybir
import concourse.tile as tile
from concourse import bacc
from concourse.bass_utils import run_bass_kernel_spmd

B, N, C, H, D = 4, 2048, 768, 12, 64
NCORES = 8
G = 2                    # head-parallel groups
CG = C // G              # 384 channels per group (6 heads)
HG = H // G              # 6 heads per core
CTG = CG // 128          # 3 head-pair slices (2 heads each)
KT = C // 128            # 6 contraction tiles (input channels)
KP = KT // 2             # 3 contraction k-tile pairs
TT = N // 128            # 16 key token tiles
TP = TT // 2             # 8 key token tile pairs
VW = 65                  # per-head V width (64 channels + sums column)
CHUNK = 512              # token chunk for qkv sweeps
NCH = N // CHUNK         # 4
SCALE = float(D) ** -0.5

WS = 32.0                # weight fp8 scale
QS = 4.0                 # q/k fp8 scale
VS = 16.0                # v fp8 scale
PK = 16.0                # pt fp8 scale

f32 = mybir.dt.float32
f32r = mybir.dt.float32r
fp8 = mybir.dt.float8e4
f16 = mybir.dt.float16
i8 = mybir.dt.int8
E4 = ml_dtypes.float8_e4m3

A8 = 8.0 / np.log(2.0)
B8 = 8.0 * (7 + np.log2(PK)) - 0.458
ESC = SCALE / (QS * QS)  # st psum holds 16x the raw logits
AF = mybir.ActivationFunctionType
ALU = mybir.AluOpType
PM = mybir.MatmulPerfMode

# exp engine per (q5, key tile): 'a'=ACT exact exp, 'd'=DVE Schraudolph
EXP_PAT = ("adadadadadadadaa", "adadadadadadadad",
           "adadadadadadadaa", "adadadadadadadad")
PT_BUFS = 8
PIPE_AHEAD = 6

_CACHE = {}


def build_nc(reps: int = 1):
    nc = bacc.Bacc("TRN2", target_bir_lowering=False, debug=False,
                   num_devices=NCORES)
    x8T = nc.dram_tensor("x8T", [C, N], fp8, kind="ExternalInput")
    xrT = nc.dram_tensor("xrT", [C, N], fp8, kind="ExternalInput")
    wqk8 = nc.dram_tensor("wqk8", [C, 2 * CG], fp8, kind="ExternalInput")
    wqkr = nc.dram_tensor("wqkr", [C, 2 * CG], fp8, kind="ExternalInput")
    wv8 = nc.dram_tensor("wv8", [C, CG], fp8, kind="ExternalInput")
    wvr = nc.dram_tensor("wvr", [C, CG], fp8, kind="ExternalInput")
    bqkW = nc.dram_tensor("bqkW", [128, 2 * CTG], f32, kind="ExternalInput")
    bqkQ = nc.dram_tensor("bqkQ", [128, 2 * CTG], f32, kind="ExternalInput")
    pwT = nc.dram_tensor("pwT", [CG, C], f32r, kind="ExternalInput")
    out = nc.dram_tensor("out", [N, C], f16, kind="ExternalOutput")
    import os
    dbg = {}
    if os.environ.get("KV2_DEBUG") == "1":
        dbg["q8"] = nc.dram_tensor("dbg_q8", [128, 2 * N], mybir.dt.uint8,
                                   kind="ExternalOutput")
        dbg["k8"] = nc.dram_tensor("dbg_k8", [128, 2 * N], mybir.dt.uint8,
                                   kind="ExternalOutput")
        dbg["vh"] = nc.dram_tensor("dbg_vh", [128, 2 * HG * 128],
                                   mybir.dt.uint8, kind="ExternalOutput")
        dbg["vl"] = nc.dram_tensor("dbg_vl", [128, 2 * HG * 128],
                                   mybir.dt.uint8, kind="ExternalOutput")
        dbg["at"] = nc.dram_tensor("dbg_at", [128, N], f32,
                                   kind="ExternalOutput")
        dbg["pt"] = nc.dram_tensor("dbg_pt", [128, 2048], mybir.dt.uint8,
                                   kind="ExternalOutput")

    with tile.TileContext(nc) as tc:
        body(nc, tc, x8T, xrT, wqk8, wqkr, wv8, wvr, bqkW, bqkQ, pwT, out,
             reps, dbg)
    nc.compile()
    return nc


def body(nc, tc, x8T, xrT, wqk8, wqkr, wv8, wvr, bqkW, bqkQ, pwT, out, reps,
         dbg=None):
    import contextlib

    loop_ctx = tc.For_i(0, reps, 1) if reps > 1 else contextlib.nullcontext()
    with loop_ctx:
        with tc.tile_pool(name="persist", bufs=1) as persist:
            # q8/k8: [128 part = 2 heads x 64ch, pair, token]; pair1 = zeros
            q8 = [persist.tile([128, 2, N], fp8, name=f"q8_{j}", tag=f"q8_{j}")
                  for j in range(CTG)]
            k8 = [persist.tile([128, 2, N], fp8, name=f"k8_{j}", tag=f"k8_{j}")
                  for j in range(CTG)]
            # vP_hi/lo[i]: key-tile-pair-interleaved V, one 128B-aligned slot
            # per head (dual-fp8 LdWeights needs the pair stride to be a
            # multiple of 128B): [128 key, pair, head, 128] with v in 0:64,
            # the sums column at 64, 65:128 unused.
            vHi = [persist.tile([128, 2, HG, 128], fp8, name=f"vH{i}",
                                tag=f"vH{i}") for i in range(TP)]
            vLo = [persist.tile([128, 2, HG, 128], fp8, name=f"vL{i}",
                                tag=f"vL{i}") for i in range(TP)]
            attnT = [persist.tile([128, N], f32r, name=f"attnT{j}",
                                  tag=f"attnT{j}") for j in range(CTG)]
            bW_sb = persist.tile([128, 2 * CTG], f32, tag="bW")
            bQ_sb = persist.tile([128, 2 * CTG], f32, tag="bQ")
            lnk_sb = persist.tile([128, 1], f32, tag="lnk")

            x8_kpn = x8T.rearrange("(k p) n -> p k n", p=128)
            xr_kpn = xrT.rearrange("(k p) n -> p k n", p=128)
            w8_kpn = wqk8.rearrange("(k p) n -> p k n", p=128)
            wr_kpn = wqkr.rearrange("(k p) n -> p k n", p=128)
            wv8_kpn = wv8.rearrange("(k p) n -> p k n", p=128)
            wvr_kpn = wvr.rearrange("(k p) n -> p k n", p=128)

            nc.vector.memset(lnk_sb[:], float(np.log(PK)))
            for j in range(CTG):
                nc.gpsimd.memset(q8[j][:, 1, :], 0.0)
                nc.gpsimd.memset(k8[j][:, 1, :], 0.0)
            # sums columns: VS in v_hi, 0 in v_lo
            for i in range(TP):
                nc.gpsimd.memset(vHi[i][:, :, :, D:D + 1], VS)
                nc.gpsimd.memset(vLo[i][:, :, :, D:D + 1], 0.0)

            def qk_evac(dst, ps, bcol, engine):
                # dst = QS*(ps/WS + bias) ; bW = WS*bias, bQ = QS*bias
                if engine == "act":
                    nc.scalar.activation(dst, ps[:], AF.Identity,
                                         bias=bQ_sb[:, bcol:bcol + 1],
                                         scale=QS / WS)
                else:
                    nc.vector.tensor_scalar(out=dst, in0=ps[:],
                                            scalar1=bW_sb[:, bcol:bcol + 1],
                                            scalar2=QS / WS,
                                            op0=ALU.add, op1=ALU.mult)

            def qk_mms(ps, col0, xR, xrR):
                """18 paired-DR matmuls: (x8+xr)@w8 + x8@wr -> [128,CHUNK]."""
                mms = []
                for xa, wa in ((xR, wqk_sb), (xrR, wqk_sb), (xR, wqkr_sb)):
                    for i in range(KP):
                        for qh in range(CHUNK // 256):
                            mms.append((
                                wa[:, 2 * i:2 * i + 2, col0:col0 + 128],
                                xa[:, 2 * i:2 * i + 2,
                                   qh * 256:qh * 256 + 256],
                                ps[:, qh * 256:qh * 256 + 256]))
                return mms

            def run_mms(mms, nfirst, nlast):
                # PSUM start zeroes the WHOLE 2KB bank, so only the first
                # matmul of a bank may carry start=True; later regions of
                # the same bank accumulate onto the zeroed bank.
                for n, (wa, xa, pd) in enumerate(mms):
                    nc.tensor.matmul(pd, wa, xa, start=(n == 0),
                                     stop=(n >= len(mms) - nlast),
                                     perf_mode=PM.DoubleRow,
                                     skip_group_check=True)

            # ---------- sweep 1: V (all heads) + k/q for pair 0 ----------
            with (
                tc.tile_pool(name="x1f", bufs=2) as x1f,
                tc.tile_pool(name="xr1f", bufs=2) as xr1f,
                tc.tile_pool(name="qkps1", bufs=2, space="PSUM") as qkps1,
                tc.tile_pool(name="vps1", bufs=3, space="PSUM") as vps1,
            ):
                wqk_sb = persist.tile([128, KT, 2 * CG], fp8, tag="wqk")
                nc.sync.dma_start(out=wqk_sb[:], in_=w8_kpn[:, :, :])
                xf0 = x1f.tile([128, KT, CHUNK], fp8, tag="xf", name="xf_s1_0")
                nc.sync.dma_start(out=xf0[:], in_=x8_kpn[:, :, 0:CHUNK])
                xr0 = xr1f.tile([128, KT, CHUNK], fp8, tag="xr", name="xr_s1_0")
                nc.sync.dma_start(out=xr0[:], in_=xr_kpn[:, :, 0:CHUNK])
                wqkr_sb = persist.tile([128, KT, 2 * CG], fp8, tag="wqkr")
                nc.sync.dma_start(out=wqkr_sb[:], in_=wr_kpn[:, :, :])
                nc.sync.dma_start(out=bW_sb[:], in_=bqkW[:, :])
                nc.sync.dma_start(out=bQ_sb[:], in_=bqkQ[:, :])
                wv_sb = persist.tile([128, KT, CG], fp8, tag="wv8")
                nc.sync.dma_start(out=wv_sb[:], in_=wv8_kpn[:, :, :])
                wr_sb = persist.tile([128, KT, CG], fp8, tag="wvr")
                nc.sync.dma_start(out=wr_sb[:], in_=wvr_kpn[:, :, :])

                xR, xrR = xf0, xr0
                for u in range(NCH):
                    lo = u * CHUNK
                    for j in range(CTG):
                        psk = qkps1.tile([128, CHUNK], f32, tag="qk",
                                         name=f"psk{u}_{j}")
                        run_mms(qk_mms(psk, CG + j * 128, xR, xrR), 2, 2)
                        qk_evac(k8[j][:, 0, lo:lo + CHUNK], psk, CTG + j,
                                "act")
                        psq = qkps1.tile([128, CHUNK], f32, tag="qk",
                                         name=f"psq{u}_{j}")
                        run_mms(qk_mms(psq, j * 128, xR, xrR), 2, 2)
                        qk_evac(q8[j][:, 0, lo:lo + CHUNK], psq, j, "vector")
                    if u + 1 < NCH:
                        nlo = lo + CHUNK
                        xR_next = x1f.tile([128, KT, CHUNK], fp8, tag="xf",
                                           name=f"xf_s1_{u + 1}")
                        nc.sync.dma_start(out=xR_next[:],
                                          in_=x8_kpn[:, :, nlo:nlo + CHUNK])
                        xr_next = xr1f.tile([128, KT, CHUNK], fp8, tag="xr",
                                            name=f"xr_s1_{u + 1}")
                        nc.sync.dma_start(out=xr_next[:],
                                          in_=xr_kpn[:, :, nlo:nlo + CHUNK])
                    else:
                        xR_next = xr_next = None
                    # V for this chunk's 4 token tiles: 3-term fp8
                    for tloc in range(CHUNK // 128):
                        t = u * (CHUNK // 128) + tloc
                        tsl = slice(tloc * 128, (tloc + 1) * 128)
                        ps = vps1.tile([128, CG], f32, tag="v",
                                       name=f"psv{t}")
                        mms = []
                        for xa, wa in ((xR, wv_sb), (xrR, wv_sb),
                                       (xR, wr_sb)):
                            for i in range(KP):
                                for hf in range(CG // 128):
                                    csl = slice(hf * 128, hf * 128 + 128)
                                    mms.append((xa[:, 2 * i:2 * i + 2, tsl],
                                                wa[:, 2 * i:2 * i + 2, csl],
                                                ps[:, csl]))
                        run_mms(mms, CG // 128, CG // 128)
                        hvh = vHi[t // 2]
                        hvl = vLo[t // 2]
                        psh = ps[:].rearrange("p (h d) -> p h d", d=D)
                        # v_hi = VS/WS * ps ; v_lo = VS/WS * ps - v_hi
                        nc.scalar.activation(hvh[:, t % 2, :, 0:D], psh,
                                             AF.Identity, scale=VS / WS)
                        nc.vector.scalar_tensor_tensor(
                            out=hvl[:, t % 2, :, 0:D], in0=psh,
                            scalar=VS / WS, in1=hvh[:, t % 2, :, 0:D],
                            op0=ALU.mult, op1=ALU.subtract)
                    xR, xrR = xR_next, xr_next

            # ---------- attention + interleaved QKV slices + proj ----------
            with (
                tc.tile_pool(name="ptpool", bufs=PT_BUFS) as ptpool,
                tc.tile_pool(name="rlpool", bufs=2) as rlpool,
                tc.tile_pool(name="bcpool", bufs=2) as bcpool,
                tc.tile_pool(name="stps", bufs=3, space="PSUM") as stps,
                tc.tile_pool(name="otps", bufs=1, space="PSUM") as otps,
            ):
                def pull(filler, n):
                    for _ in range(n):
                        if filler is None:
                            return
                        try:
                            next(filler)
                        except StopIteration:
                            return

                it_state = {"it": 0}

                def attention_pair(j, filler=None, budget=None):
                    if budget is None:
                        budget = lambda it: 3
                    NQ5 = N // 512
                    seq = [(q5, t) for q5 in range(NQ5) for t in range(TT)]
                    ots = {}

                    def st_exp(q5, t):
                        qlo = q5 * 512
                        st = stps.tile([128, 1024], f32, tag="st",
                                       name=f"st_{j}_{q5}_{t}")
                        for h in range(2):
                            hp = slice(64 * h, 64 * h + 64)
                            for qh in range(2):
                                qsl = slice(qlo + qh * 256,
                                            qlo + qh * 256 + 256)
                                nc.tensor.matmul(
                                    st[:, h * 512 + qh * 256:
                                       h * 512 + qh * 256 + 256],
                                    k8[j][hp, :, t * 128:(t + 1) * 128],
                                    q8[j][hp, :, qsl],
                                    start=(qh == 0), stop=True,
                                    perf_mode=PM.DoubleRow,
                                    skip_group_check=True)
                        return st

                    def emit_exp(q5, t, st, ptP):
                        if EXP_PAT[q5][t] == "a":
                            nc.scalar.activation(
                                ptP.bitcast(fp8)[:, t % 2, :], st[:], AF.Exp,
                                bias=lnk_sb[:, 0:1], scale=ESC)
                        else:
                            nc.vector.tensor_scalar(
                                out=ptP[:, t % 2, :], in0=st[:],
                                scalar1=A8 * ESC, scalar2=B8,
                                op0=ALU.mult, op1=ALU.add)

                    def emit_pv(q5, i, ptP):
                        ot = ots[q5]
                        pt8 = ptP.bitcast(fp8).rearrange(
                            "p two (h q) -> p two h q", h=2)
                        for h in range(2):
                            for qh in range(2):
                                osl = slice(h * 512 + qh * 256,
                                            h * 512 + qh * 256 + 256)
                                rhs = pt8[:, :, h, qh * 256:qh * 256 + 256]
                                nc.tensor.matmul(
                                    ot[:, osl],
                                    vHi[i][:, :, 2 * j + h, 0:VW], rhs,
                                    start=(i == 0 and qh == 0), stop=False,
                                    perf_mode=PM.DoubleRow,
                                    skip_group_check=True)
                                nc.tensor.matmul(
                                    ot[:, osl],
                                    vLo[i][:, :, 2 * j + h, 0:VW], rhs,
                                    start=False, stop=(i == TP - 1),
                                    perf_mode=PM.DoubleRow,
                                    skip_group_check=True)

                    osbs = {}

                    def emit_evac_copy(q5):
                        ot = ots.pop(q5)
                        # high-priority copy frees the psum banks fast
                        osb = rlpool.tile([VW, 1024], f32, tag="osb",
                                          name=f"osb_{j}_{q5}")
                        with tc.high_priority():
                            nc.scalar.activation(osb[:], ot[:], AF.Copy)
                        osbs[q5] = osb

                    def emit_evac_norm(q5):
                        qlo = q5 * 512
                        osb = osbs.pop(q5)
                        rl = rlpool.tile([1, 1024], f32, tag="rl",
                                         name=f"rl_{j}_{q5}")
                        nc.vector.reciprocal(rl[0:1, :], osb[D:D + 1, :])
                        bc = bcpool.tile([64, 1024], f32, tag="bc",
                                         name=f"bc_{j}_{q5}")
                        nc.gpsimd.partition_broadcast(bc[:], rl[0:1, :])
                        for h in range(2):
                            nc.gpsimd.tensor_tensor(
                                out=attnT[j][64 * h:64 * h + 64,
                                             qlo:qlo + 512],
                                in0=osb[0:D, h * 512:h * 512 + 512],
                                in1=bc[:, h * 512:h * 512 + 512],
                                op=ALU.mult)

                    def new_pt(q5, i):
                        return ptpool.tile([128, 2, 1024], i8, tag="pt",
                                           name=f"pt_{j}_{q5}_{i}")

                    pts = {}
                    pending = []
                    for idx in range(PIPE_AHEAD):
                        q5, t = seq[idx]
                        if t % 2 == 0:
                            pts[(q5, t // 2)] = new_pt(q5, t // 2)
                        emit_exp(q5, t, st_exp(q5, t), pts[(q5, t // 2)])
                    for it, (q5, t) in enumerate(seq):
                        if t == 0:
                            ots[q5] = otps.tile([VW, 1024], f32, tag="ot",
                                                name=f"ot_{j}_{q5}")
                        # exp pipeline first so PE/engine queues never sit
                        # behind a blocked PV or a waiting recip
                        if it + PIPE_AHEAD < len(seq):
                            q5n, tn = seq[it + PIPE_AHEAD]
                            if tn % 2 == 0:
                                pts[(q5n, tn // 2)] = new_pt(q5n, tn // 2)
                            emit_exp(q5n, tn, st_exp(q5n, tn),
                                     pts[(q5n, tn // 2)])
                        if t % 2 == 1:
                            emit_pv(q5, t // 2, pts.pop((q5, t // 2)))
                        if t == TT - 1:
                            emit_evac_copy(q5)
                            pending.append((q5, t))
                        # deferred: the recip/bcast/mults of q5 run 4 tiles
                        # into q5+1 so the recip never heads the DVE queue
                        if pending and (t == 4 or it == len(seq) - 1):
                            emit_evac_norm(pending.pop(0)[0])
                        it_state["it"] = it
                        pull(filler, budget(it))
                    while pending:
                        emit_evac_norm(pending.pop(0)[0])

                import os
                phases = os.environ.get("KV2_PHASES", "s1,p0,p1,p2,proj").split(",")
                for j in range(CTG - 1):
                    if f"p{j}" in phases:
                        attention_pair(j)

                if "p2" in phases:
                    attention_pair(CTG - 1)

            # ------------------- projection tail phase -------------------
            with (
                tc.tile_pool(name="outsb", bufs=3) as outsb,
                tc.tile_pool(name="prps", bufs=2, space="PSUM") as prps,
            ):
                if True:
                    pwT_kpn = pwT.rearrange("(k p) n -> p k n", p=128)
                    pwf = persist.tile([128, CTG, C], f32r, tag="pwf")
                    nc.sync.dma_start(out=pwf[:], in_=pwT_kpn[:, :, :])

                    def proj_tok_tile(tt):
                        osb = outsb.tile([128, C], f16, tag="osb",
                                         name=f"osb_p{tt}")
                        for half in range(2):
                            ps = prps.tile([128, C // 2], f32, tag="pr",
                                           name=f"prps_{tt}_{half}")
                            for k in range(CTG):
                                nc.tensor.matmul(
                                    ps[:],
                                    attnT[k][:, tt * 128:(tt + 1) * 128],
                                    pwf[:, k, half * (C // 2):
                                        (half + 1) * (C // 2)],
                                    start=(k == 0), stop=(k == CTG - 1))
                                yield
                            dst = osb[:, half * (C // 2):(half + 1) * (C // 2)]
                            if half == 0:
                                nc.scalar.activation(dst, ps[:], AF.Copy)
                            else:
                                nc.vector.tensor_copy(dst, ps[:])
                        nc.sync.dma_start(
                            out=out[tt * 128:(tt + 1) * 128, :], in_=osb[:])

                    rest = range(TT) if "proj" in phases else []
                    for tt in rest:
                        for _ in proj_tok_tile(tt):
                            pass
                    if dbg:
                        nc.sync.dma_start(out=dbg["q8"][:, :],
                                          in_=q8[0].bitcast(mybir.dt.uint8)[:, :, :])
                        nc.sync.dma_start(out=dbg["k8"][:, :],
                                          in_=k8[0].bitcast(mybir.dt.uint8)[:, :, :])
                        nc.sync.dma_start(out=dbg["vh"][:, :],
                                          in_=vHi[0].bitcast(mybir.dt.uint8)[:, :, :, :])
                        nc.sync.dma_start(out=dbg["vl"][:, :],
                                          in_=vLo[0].bitcast(mybir.dt.uint8)[:, :, :, :])
                        nc.sync.dma_start(out=dbg["at"][:, :],
                                          in_=attnT[0].bitcast(f32)[:, :])


def _prepare_inputs(x, qkv_w, qkv_b, proj_w, proj_b):
    """Host-side shard preparation (numpy quantize/reshape/transpose)."""
    x = np.asarray(x, dtype=np.float32)
    x8 = x.astype(E4)
    xr = (x - x8.astype(np.float32)).astype(E4)
    in_maps = []
    for core in range(NCORES):
        b, s = core // G, core % G
        rq = slice(s * CG, (s + 1) * CG)
        rk = slice(C + s * CG, C + (s + 1) * CG)
        rv = slice(2 * C + s * CG, 2 * C + (s + 1) * CG)
        wqk = np.concatenate([qkv_w[rq], qkv_w[rk]], 0).T  # [768, 768]
        wqk8 = (WS * wqk).astype(E4)
        wqkr = (WS * wqk - wqk8.astype(np.float32)).astype(E4)
        wv = qkv_w[rv].T  # [768, 384]
        wv8 = (WS * wv).astype(E4)
        wvr = (WS * wv - wv8.astype(np.float32)).astype(E4)
        bvec = np.concatenate([qkv_b[rq], qkv_b[rk]])
        bmat = bvec.reshape(2 * CTG, 128).T
        pwT = np.ascontiguousarray(proj_w.T[s * CG:(s + 1) * CG, :]).astype(
            np.float32)
        in_maps.append({
            "x8T": np.ascontiguousarray(x8[b].T),
            "xrT": np.ascontiguousarray(xr[b].T),
            "wqk8": np.ascontiguousarray(wqk8),
            "wqkr": np.ascontiguousarray(wqkr),
            "wv8": np.ascontiguousarray(wv8),
            "wvr": np.ascontiguousarray(wvr),
            "bqkW": np.ascontiguousarray(WS * bmat).astype(np.float32),
            "bqkQ": np.ascontiguousarray(QS * bmat).astype(np.float32),
            "pwT": pwT,
        })
    return in_maps


def kernel(x, qkv_w, qkv_b, proj_w, proj_b):
    x = np.asarray(x, dtype=np.float32)
    qkv_w = np.asarray(qkv_w, dtype=np.float32)
    qkv_b = np.asarray(qkv_b, dtype=np.float32)
    proj_w = np.asarray(proj_w, dtype=np.float32)
    proj_b = np.asarray(proj_b, dtype=np.float32)

    if "nc" not in _CACHE:
        _CACHE["nc"] = build_nc(reps=1)
    nc = _CACHE["nc"]

    in_maps = _prepare_inputs(x, qkv_w, qkv_b, proj_w, proj_b)
    res = run_bass_kernel_spmd(nc, in_maps, list(range(NCORES)))

    # host-side bias fold: v-bias through proj + proj bias
    pb_total = (proj_b + qkv_b[2 * C:] @ proj_w.T).astype(np.float32)
    out = np.empty((B, N, C), dtype=np.float32)
    for b in range(B):
        out[b] = res.results[G * b]["out"].astype(np.float32)
        out[b] += res.results[G * b + 1]["out"].astype(np.float32)
        out[b] += pb_total
    return out


# revision 3
# speedup vs baseline: 2.4178x; 2.4178x over previous
"""Multi-head attention on 8 Trainium2 NeuronCores — fp8-DoubleRow edition.

Problem: x[4, 2048, 768] -> qkv (12 heads, d=64) -> softmax attention -> proj.

Sharding: data-parallel over batch (4) x tensor-parallel over heads (2
groups of 6 heads) -> 8 shards; the host adds the two partial projections
per batch plus the (qkv v-bias + proj bias) fold (pure numpy adds).

All heavy matmuls run as fp8e4m3 MatmulPerfMode.DoubleRow (0.5 cycles/row;
pairing two 128-deep contraction tiles per matmul halves the row count
again), taking per-core PE busy from ~228us (f32r baseline) to ~133us. The
bottleneck becomes the ACT/DVE elementwise wall: softmax exp of 25.2M
logits/core plus all PSUM evacuation (Pool/GPSIMD cannot access PSUM and
DMA cannot read PSUM, so exactly two engines can touch PSUM). The exp is
split ACT (exact table exp -> fp8, ln16 bias) / DVE (Schraudolph integer
exp: e4m3 bits = int8(A8*x + B8), ~3% sawtooth) roughly 34:30 per pair;
Pool handles the softmax normalization (broadcast + multiply, all SBUF).

Numerics (e4m3 denormals start at 2^-6, so every small tensor is quantized
at a power-of-2 scale, undone downstream for free): weights at 32x (undone
in the evac affine), q/k at 4x (undone in the exp argument), v at 16x with
the PV sums column holding 16.0 so normalization cancels everything.
Accuracy is bought with residual terms where it is cheap: q/k = (x8 + xr) @
w8 + x8 @ wr (host-prepared fp8 residual tensors), and V feeds PV as TWO
fp8 tensors v_hi + v_lo with two DoubleRow accumulation chains (~bf16
quality at fp8 speed). pt is fp8 at 16x. proj stays f32r (fp8 would cost
2.4e-2 rel err alone; 3-term fp8 proj is exactly cost-neutral vs f32r with
an odd k-tile count). Output DMA is fp16, upcast host-side.

Hardware gotchas found on the way: dual-fp8 LdWeights requires the pair
stride to be a multiple of 128B (V is stored in 128B-aligned per-head
slots); a PSUM start=True zeroes the WHOLE 2KB bank, so exactly one matmul
per bank carries start; the QK DoubleRow puts the 64-channel contraction in
pair0 and zeros in pair1 so no partition remap of q/k is ever needed.

Schedule: sweep1 (PE-bound, ~44us) computes q/k for all pairs + V; three
fillerless attention pairs (~45us each, exp-bound, 3 st psum bufs so QK
never sits on the exp critical path; the normalization chain is deferred 4
tiles so the DVE reciprocal never heads the queue); f32r projection tail.
Cost-model span 204.6us (baseline 254.1us); measured rel err 1.64e-2
(gate 2e-2).
"""

import sys

sys.path.insert(0, "/opt/trn_rl_repo")

import numpy as np
import ml_dtypes

import concourse.bass as bass
import concourse.mybir as mybir
import concourse.tile as tile
from concourse import bacc
from concourse.bass_utils import run_bass_kernel_spmd

B, N, C, H, D = 4, 2048, 768, 12, 64
NCORES = 8
G = 2                    # head-parallel groups
CG = C // G              # 384 channels per group (6 heads)
HG = H // G              # 6 heads per core
CTG = CG // 128          # 3 head-pair slices (2 heads each)
KT = C // 128            # 6 contraction tiles (input channels)
KP = KT // 2             # 3 contraction k-tile pairs
TT = N // 128            # 16 key token tiles
TP = TT // 2             # 8 key token tile pairs
VW = 65                  # per-head V width (64 channels + sums column)
CHUNK = 512              # token chunk for qkv sweeps
NCH = N // CHUNK         # 4
SCALE = float(D) ** -0.5

WS = 32.0                # weight fp8 scale
QS = 4.0                 # q/k fp8 scale
VS = 16.0                # v fp8 scale
PK = 16.0                # pt fp8 scale

f32 = mybir.dt.float32
f32r = mybir.dt.float32r
fp8 = mybir.dt.float8e4
f16 = mybir.dt.float16
i8 = mybir.dt.int8
E4 = ml_dtypes.float8_e4m3

A8 = 8.0 / np.log(2.0)
B8 = 8.0 * (7 + np.log2(PK)) - 0.458
ESC = SCALE / (QS * QS)  # st psum holds 16x the raw logits
AF = mybir.ActivationFunctionType
ALU = mybir.AluOpType
PM = mybir.MatmulPerfMode

# exp engine per (q5, key tile): 'a'=ACT exact exp, 'd'=DVE Schraudolph
EXP_PAT = ("adadadadadadadaa", "adadadadadadadad",
           "adadadadadadadad", "adadadadadadadad")
PT_BUFS = 10
PIPE_AHEAD = 8

_CACHE = {}


def build_nc(reps: int = 1):
    nc = bacc.Bacc("TRN2", target_bir_lowering=False, debug=False,
                   num_devices=NCORES)
    x8T = nc.dram_tensor("x8T", [C, N], fp8, kind="ExternalInput")
    xrT = nc.dram_tensor("xrT", [C, N], fp8, kind="ExternalInput")
    wqk8 = nc.dram_tensor("wqk8", [C, 2 * CG], fp8, kind="ExternalInput")
    wqkr = nc.dram_tensor("wqkr", [C, 2 * CG], fp8, kind="ExternalInput")
    wv8 = nc.dram_tensor("wv8", [C, CG], fp8, kind="ExternalInput")
    wvr = nc.dram_tensor("wvr", [C, CG], fp8, kind="ExternalInput")
    bqkW = nc.dram_tensor("bqkW", [128, 2 * CTG], f32, kind="ExternalInput")
    bqkQ = nc.dram_tensor("bqkQ", [128, 2 * CTG], f32, kind="ExternalInput")
    pwT = nc.dram_tensor("pwT", [CG, C], f32r, kind="ExternalInput")
    out = nc.dram_tensor("out", [N, C], f16, kind="ExternalOutput")
    import os
    dbg = {}
    if os.environ.get("KV2_DEBUG") == "1":
        dbg["q8"] = nc.dram_tensor("dbg_q8", [128, 2 * N], mybir.dt.uint8,
                                   kind="ExternalOutput")
        dbg["k8"] = nc.dram_tensor("dbg_k8", [128, 2 * N], mybir.dt.uint8,
                                   kind="ExternalOutput")
        dbg["vh"] = nc.dram_tensor("dbg_vh", [128, 2 * HG * 128],
                                   mybir.dt.uint8, kind="ExternalOutput")
        dbg["vl"] = nc.dram_tensor("dbg_vl", [128, 2 * HG * 128],
                                   mybir.dt.uint8, kind="ExternalOutput")
        dbg["at"] = nc.dram_tensor("dbg_at", [128, N], f32,
                                   kind="ExternalOutput")
        dbg["pt"] = nc.dram_tensor("dbg_pt", [128, 2048], mybir.dt.uint8,
                                   kind="ExternalOutput")

    with tile.TileContext(nc) as tc:
        body(nc, tc, x8T, xrT, wqk8, wqkr, wv8, wvr, bqkW, bqkQ, pwT, out,
             reps, dbg)
    nc.compile()
    return nc


def body(nc, tc, x8T, xrT, wqk8, wqkr, wv8, wvr, bqkW, bqkQ, pwT, out, reps,
         dbg=None):
    import contextlib

    loop_ctx = tc.For_i(0, reps, 1) if reps > 1 else contextlib.nullcontext()
    with loop_ctx:
        with tc.tile_pool(name="persist", bufs=1) as persist:
            # q8/k8: [128 part = 2 heads x 64ch, pair, token]; pair1 = zeros
            q8 = [persist.tile([128, 2, N], fp8, name=f"q8_{j}", tag=f"q8_{j}")
                  for j in range(CTG)]
            k8 = [persist.tile([128, 2, N], fp8, name=f"k8_{j}", tag=f"k8_{j}")
                  for j in range(CTG)]
            # vP_hi/lo[i]: key-tile-pair-interleaved V, one 128B-aligned slot
            # per head (dual-fp8 LdWeights needs the pair stride to be a
            # multiple of 128B): [128 key, pair, head, 128] with v in 0:64,
            # the sums column at 64, 65:128 unused.
            vHi = [persist.tile([128, 2, HG, 128], fp8, name=f"vH{i}",
                                tag=f"vH{i}") for i in range(TP)]
            vLo = [persist.tile([128, 2, HG, 128], fp8, name=f"vL{i}",
                                tag=f"vL{i}") for i in range(TP)]
            attnT = [persist.tile([128, N], f32r, name=f"attnT{j}",
                                  tag=f"attnT{j}") for j in range(CTG)]
            bW_sb = persist.tile([128, 2 * CTG], f32, tag="bW")
            bQ_sb = persist.tile([128, 2 * CTG], f32, tag="bQ")
            lnk_sb = persist.tile([128, 1], f32, tag="lnk")

            x8_kpn = x8T.rearrange("(k p) n -> p k n", p=128)
            xr_kpn = xrT.rearrange("(k p) n -> p k n", p=128)
            w8_kpn = wqk8.rearrange("(k p) n -> p k n", p=128)
            wr_kpn = wqkr.rearrange("(k p) n -> p k n", p=128)
            wv8_kpn = wv8.rearrange("(k p) n -> p k n", p=128)
            wvr_kpn = wvr.rearrange("(k p) n -> p k n", p=128)

            nc.vector.memset(lnk_sb[:], float(np.log(PK)))
            for j in range(CTG):
                nc.gpsimd.memset(q8[j][:, 1, :], 0.0)
                nc.gpsimd.memset(k8[j][:, 1, :], 0.0)
            # sums columns: VS in v_hi, 0 in v_lo
            for i in range(TP):
                nc.gpsimd.memset(vHi[i][:, :, :, D:D + 1], VS)
                nc.gpsimd.memset(vLo[i][:, :, :, D:D + 1], 0.0)

            def qk_evac(dst, ps, bcol, engine):
                # dst = QS*(ps/WS + bias) ; bW = WS*bias, bQ = QS*bias
                if engine == "act":
                    nc.scalar.activation(dst, ps[:], AF.Identity,
                                         bias=bQ_sb[:, bcol:bcol + 1],
                                         scale=QS / WS)
                else:
                    nc.vector.tensor_scalar(out=dst, in0=ps[:],
                                            scalar1=bW_sb[:, bcol:bcol + 1],
                                            scalar2=QS / WS,
                                            op0=ALU.add, op1=ALU.mult)

            def qk_mms(ps, col0, xR, xrR):
                """18 paired-DR matmuls: (x8+xr)@w8 + x8@wr -> [128,CHUNK]."""
                mms = []
                for xa, wa in ((xR, wqk_sb), (xrR, wqk_sb), (xR, wqkr_sb)):
                    for i in range(KP):
                        for qh in range(CHUNK // 256):
                            mms.append((
                                wa[:, 2 * i:2 * i + 2, col0:col0 + 128],
                                xa[:, 2 * i:2 * i + 2,
                                   qh * 256:qh * 256 + 256],
                                ps[:, qh * 256:qh * 256 + 256]))
                return mms

            def run_mms(mms, nfirst, nlast):
                # PSUM start zeroes the WHOLE 2KB bank, so only the first
                # matmul of a bank may carry start=True; later regions of
                # the same bank accumulate onto the zeroed bank.
                for n, (wa, xa, pd) in enumerate(mms):
                    nc.tensor.matmul(pd, wa, xa, start=(n == 0),
                                     stop=(n >= len(mms) - nlast),
                                     perf_mode=PM.DoubleRow,
                                     skip_group_check=True)

            # ---------- sweep 1: V (all heads) + k/q for pair 0 ----------
            with (
                tc.tile_pool(name="x1f", bufs=2) as x1f,
                tc.tile_pool(name="xr1f", bufs=2) as xr1f,
                tc.tile_pool(name="qkps1", bufs=2, space="PSUM") as qkps1,
                tc.tile_pool(name="vps1", bufs=3, space="PSUM") as vps1,
            ):
                wqk_sb = persist.tile([128, KT, 2 * CG], fp8, tag="wqk")
                nc.sync.dma_start(out=wqk_sb[:], in_=w8_kpn[:, :, :])
                xf0 = x1f.tile([128, KT, CHUNK], fp8, tag="xf", name="xf_s1_0")
                nc.sync.dma_start(out=xf0[:], in_=x8_kpn[:, :, 0:CHUNK])
                xr0 = xr1f.tile([128, KT, CHUNK], fp8, tag="xr", name="xr_s1_0")
                nc.sync.dma_start(out=xr0[:], in_=xr_kpn[:, :, 0:CHUNK])
                wqkr_sb = persist.tile([128, KT, 2 * CG], fp8, tag="wqkr")
                nc.sync.dma_start(out=wqkr_sb[:], in_=wr_kpn[:, :, :])
                nc.sync.dma_start(out=bW_sb[:], in_=bqkW[:, :])
                nc.sync.dma_start(out=bQ_sb[:], in_=bqkQ[:, :])
                wv_sb = persist.tile([128, KT, CG], fp8, tag="wv8")
                nc.sync.dma_start(out=wv_sb[:], in_=wv8_kpn[:, :, :])
                wr_sb = persist.tile([128, KT, CG], fp8, tag="wvr")
                nc.sync.dma_start(out=wr_sb[:], in_=wvr_kpn[:, :, :])

                xR, xrR = xf0, xr0
                for u in range(NCH):
                    lo = u * CHUNK
                    for j in range(CTG):
                        psk = qkps1.tile([128, CHUNK], f32, tag="qk",
                                         name=f"psk{u}_{j}")
                        run_mms(qk_mms(psk, CG + j * 128, xR, xrR), 2, 2)
                        qk_evac(k8[j][:, 0, lo:lo + CHUNK], psk, CTG + j,
                                "act")
                        psq = qkps1.tile([128, CHUNK], f32, tag="qk",
                                         name=f"psq{u}_{j}")
                        run_mms(qk_mms(psq, j * 128, xR, xrR), 2, 2)
                        qk_evac(q8[j][:, 0, lo:lo + CHUNK], psq, j, "vector")
                    if u + 1 < NCH:
                        nlo = lo + CHUNK
                        xR_next = x1f.tile([128, KT, CHUNK], fp8, tag="xf",
                                           name=f"xf_s1_{u + 1}")
                        nc.sync.dma_start(out=xR_next[:],
                                          in_=x8_kpn[:, :, nlo:nlo + CHUNK])
                        xr_next = xr1f.tile([128, KT, CHUNK], fp8, tag="xr",
                                            name=f"xr_s1_{u + 1}")
                        nc.sync.dma_start(out=xr_next[:],
                                          in_=xr_kpn[:, :, nlo:nlo + CHUNK])
                    else:
                        xR_next = xr_next = None
                    # V for this chunk's 4 token tiles: 3-term fp8
                    for tloc in range(CHUNK // 128):
                        t = u * (CHUNK // 128) + tloc
                        tsl = slice(tloc * 128, (tloc + 1) * 128)
                        ps = vps1.tile([128, CG], f32, tag="v",
                                       name=f"psv{t}")
                        mms = []
                        for xa, wa in ((xR, wv_sb), (xrR, wv_sb),
                                       (xR, wr_sb)):
                            for i in range(KP):
                                for hf in range(CG // 128):
                                    csl = slice(hf * 128, hf * 128 + 128)
                                    mms.append((xa[:, 2 * i:2 * i + 2, tsl],
                                                wa[:, 2 * i:2 * i + 2, csl],
                                                ps[:, csl]))
                        run_mms(mms, CG // 128, CG // 128)
                        hvh = vHi[t // 2]
                        hvl = vLo[t // 2]
                        psh = ps[:].rearrange("p (h d) -> p h d", d=D)
                        # v_hi = VS/WS * ps ; v_lo = VS/WS * ps - v_hi
                        nc.scalar.activation(hvh[:, t % 2, :, 0:D], psh,
                                             AF.Identity, scale=VS / WS)
                        nc.vector.scalar_tensor_tensor(
                            out=hvl[:, t % 2, :, 0:D], in0=psh,
                            scalar=VS / WS, in1=hvh[:, t % 2, :, 0:D],
                            op0=ALU.mult, op1=ALU.subtract)
                    xR, xrR = xR_next, xr_next

            # ---------- attention + interleaved QKV slices + proj ----------
            with (
                tc.tile_pool(name="ptpool", bufs=PT_BUFS) as ptpool,
                tc.tile_pool(name="rlpool", bufs=2) as rlpool,
                tc.tile_pool(name="bcpool", bufs=2) as bcpool,
                tc.tile_pool(name="stps", bufs=3, space="PSUM") as stps,
                tc.tile_pool(name="otps", bufs=1, space="PSUM") as otps,
            ):
                def pull(filler, n):
                    for _ in range(n):
                        if filler is None:
                            return
                        try:
                            next(filler)
                        except StopIteration:
                            return

                it_state = {"it": 0}

                def attention_pair(j, filler=None, budget=None):
                    if budget is None:
                        budget = lambda it: 3
                    NQ5 = N // 512
                    seq = [(q5, t) for q5 in range(NQ5) for t in range(TT)]
                    ots = {}

                    def st_exp(q5, t):
                        qlo = q5 * 512
                        st = stps.tile([128, 1024], f32, tag="st",
                                       name=f"st_{j}_{q5}_{t}")
                        for h in range(2):
                            hp = slice(64 * h, 64 * h + 64)
                            for qh in range(2):
                                qsl = slice(qlo + qh * 256,
                                            qlo + qh * 256 + 256)
                                nc.tensor.matmul(
                                    st[:, h * 512 + qh * 256:
                                       h * 512 + qh * 256 + 256],
                                    k8[j][hp, :, t * 128:(t + 1) * 128],
                                    q8[j][hp, :, qsl],
                                    start=(qh == 0), stop=True,
                                    perf_mode=PM.DoubleRow,
                                    skip_group_check=True)
                        return st

                    def emit_exp(q5, t, st, ptP):
                        if EXP_PAT[q5][t] == "a":
                            nc.scalar.activation(
                                ptP.bitcast(fp8)[:, t % 2, :], st[:], AF.Exp,
                                bias=lnk_sb[:, 0:1], scale=ESC)
                        else:
                            nc.vector.tensor_scalar(
                                out=ptP[:, t % 2, :], in0=st[:],
                                scalar1=A8 * ESC, scalar2=B8,
                                op0=ALU.mult, op1=ALU.add)

                    def emit_pv(q5, i, ptP):
                        ot = ots[q5]
                        pt8 = ptP.bitcast(fp8).rearrange(
                            "p two (h q) -> p two h q", h=2)
                        for h in range(2):
                            for qh in range(2):
                                osl = slice(h * 512 + qh * 256,
                                            h * 512 + qh * 256 + 256)
                                rhs = pt8[:, :, h, qh * 256:qh * 256 + 256]
                                nc.tensor.matmul(
                                    ot[:, osl],
                                    vHi[i][:, :, 2 * j + h, 0:VW], rhs,
                                    start=(i == 0 and qh == 0), stop=False,
                                    perf_mode=PM.DoubleRow,
                                    skip_group_check=True)
                                nc.tensor.matmul(
                                    ot[:, osl],
                                    vLo[i][:, :, 2 * j + h, 0:VW], rhs,
                                    start=False, stop=(i == TP - 1),
                                    perf_mode=PM.DoubleRow,
                                    skip_group_check=True)

                    osbs = {}

                    def emit_evac_copy(q5):
                        ot = ots.pop(q5)
                        # high-priority copy frees the psum banks fast
                        osb = rlpool.tile([VW, 1024], f32, tag="osb",
                                          name=f"osb_{j}_{q5}")
                        with tc.high_priority():
                            nc.scalar.activation(osb[:], ot[:], AF.Copy)
                        osbs[q5] = osb

                    def emit_evac_norm(q5):
                        qlo = q5 * 512
                        osb = osbs.pop(q5)
                        rl = rlpool.tile([1, 1024], f32, tag="rl",
                                         name=f"rl_{j}_{q5}")
                        nc.vector.reciprocal(rl[0:1, :], osb[D:D + 1, :])
                        bc = bcpool.tile([64, 1024], f32, tag="bc",
                                         name=f"bc_{j}_{q5}")
                        nc.gpsimd.partition_broadcast(bc[:], rl[0:1, :])
                        for h in range(2):
                            nc.gpsimd.tensor_tensor(
                                out=attnT[j][64 * h:64 * h + 64,
                                             qlo:qlo + 512],
                                in0=osb[0:D, h * 512:h * 512 + 512],
                                in1=bc[:, h * 512:h * 512 + 512],
                                op=ALU.mult)

                    def new_pt(q5, i):
                        return ptpool.tile([128, 2, 1024], i8, tag="pt",
                                           name=f"pt_{j}_{q5}_{i}")

                    pts = {}
                    pending = []
                    for idx in range(PIPE_AHEAD):
                        q5, t = seq[idx]
                        if t % 2 == 0:
                            pts[(q5, t // 2)] = new_pt(q5, t // 2)
                        emit_exp(q5, t, st_exp(q5, t), pts[(q5, t // 2)])
                    for it, (q5, t) in enumerate(seq):
                        if t == 0:
                            ots[q5] = otps.tile([VW, 1024], f32, tag="ot",
                                                name=f"ot_{j}_{q5}")
                        # exp pipeline first so PE/engine queues never sit
                        # behind a blocked PV or a waiting recip
                        if it + PIPE_AHEAD < len(seq):
                            q5n, tn = seq[it + PIPE_AHEAD]
                            if tn % 2 == 0:
                                pts[(q5n, tn // 2)] = new_pt(q5n, tn // 2)
                            emit_exp(q5n, tn, st_exp(q5n, tn),
                                     pts[(q5n, tn // 2)])
                        if t % 2 == 1:
                            emit_pv(q5, t // 2, pts.pop((q5, t // 2)))
                        if t == TT - 1:
                            emit_evac_copy(q5)
                            pending.append((q5, t))
                        # deferred: the recip/bcast/mults of q5 run 4 tiles
                        # into q5+1 so the recip never heads the DVE queue
                        if pending and (t == 4 or it == len(seq) - 1):
                            emit_evac_norm(pending.pop(0)[0])
                        it_state["it"] = it
                        pull(filler, budget(it))
                    while pending:
                        emit_evac_norm(pending.pop(0)[0])

                import os
                phases = os.environ.get("KV2_PHASES", "s1,p0,p1,p2,proj").split(",")
                for j in range(CTG - 1):
                    if f"p{j}" in phases:
                        attention_pair(j)

                if "p2" in phases:
                    attention_pair(CTG - 1)

            # ------------------- projection tail phase -------------------
            with (
                tc.tile_pool(name="outsb", bufs=3) as outsb,
                tc.tile_pool(name="prps", bufs=2, space="PSUM") as prps,
            ):
                if True:
                    pwT_kpn = pwT.rearrange("(k p) n -> p k n", p=128)
                    pwf = persist.tile([128, CTG, C], f32r, tag="pwf")
                    nc.sync.dma_start(out=pwf[:], in_=pwT_kpn[:, :, :])

                    def proj_tok_tile(tt):
                        osb = outsb.tile([128, C], f16, tag="osb",
                                         name=f"osb_p{tt}")
                        for half in range(2):
                            ps = prps.tile([128, C // 2], f32, tag="pr",
                                           name=f"prps_{tt}_{half}")
                            for k in range(CTG):
                                nc.tensor.matmul(
                                    ps[:],
                                    attnT[k][:, tt * 128:(tt + 1) * 128],
                                    pwf[:, k, half * (C // 2):
                                        (half + 1) * (C // 2)],
                                    start=(k == 0), stop=(k == CTG - 1))
                                yield
                            dst = osb[:, half * (C // 2):(half + 1) * (C // 2)]
                            if half == 0:
                                nc.scalar.activation(dst, ps[:], AF.Copy)
                            else:
                                nc.vector.tensor_copy(dst, ps[:])
                        nc.sync.dma_start(
                            out=out[tt * 128:(tt + 1) * 128, :], in_=osb[:])

                    rest = range(TT) if "proj" in phases else []
                    for tt in rest:
                        for _ in proj_tok_tile(tt):
                            pass
                    if dbg:
                        nc.sync.dma_start(out=dbg["q8"][:, :],
                                          in_=q8[0].bitcast(mybir.dt.uint8)[:, :, :])
                        nc.sync.dma_start(out=dbg["k8"][:, :],
                                          in_=k8[0].bitcast(mybir.dt.uint8)[:, :, :])
                        nc.sync.dma_start(out=dbg["vh"][:, :],
                                          in_=vHi[0].bitcast(mybir.dt.uint8)[:, :, :, :])
                        nc.sync.dma_start(out=dbg["vl"][:, :],
                                          in_=vLo[0].bitcast(mybir.dt.uint8)[:, :, :, :])
                        nc.sync.dma_start(out=dbg["at"][:, :],
                                          in_=attnT[0].bitcast(f32)[:, :])


def _prepare_inputs(x, qkv_w, qkv_b, proj_w, proj_b):
    """Host-side shard preparation (numpy quantize/reshape/transpose)."""
    x = np.asarray(x, dtype=np.float32)
    x8 = x.astype(E4)
    xr = (x - x8.astype(np.float32)).astype(E4)
    in_maps = []
    for core in range(NCORES):
        b, s = core // G, core % G
        rq = slice(s * CG, (s + 1) * CG)
        rk = slice(C + s * CG, C + (s + 1) * CG)
        rv = slice(2 * C + s * CG, 2 * C + (s + 1) * CG)
        wqk = np.concatenate([qkv_w[rq], qkv_w[rk]], 0).T  # [768, 768]
        wqk8 = (WS * wqk).astype(E4)
        wqkr = (WS * wqk - wqk8.astype(np.float32)).astype(E4)
        wv = qkv_w[rv].T  # [768, 384]
        wv8 = (WS * wv).astype(E4)
        wvr = (WS * wv - wv8.astype(np.float32)).astype(E4)
        bvec = np.concatenate([qkv_b[rq], qkv_b[rk]])
        bmat = bvec.reshape(2 * CTG, 128).T
        pwT = np.ascontiguousarray(proj_w.T[s * CG:(s + 1) * CG, :]).astype(
            np.float32)
        in_maps.append({
            "x8T": np.ascontiguousarray(x8[b].T),
            "xrT": np.ascontiguousarray(xr[b].T),
            "wqk8": np.ascontiguousarray(wqk8),
            "wqkr": np.ascontiguousarray(wqkr),
            "wv8": np.ascontiguousarray(wv8),
            "wvr": np.ascontiguousarray(wvr),
            "bqkW": np.ascontiguousarray(WS * bmat).astype(np.float32),
            "bqkQ": np.ascontiguousarray(QS * bmat).astype(np.float32),
            "pwT": pwT,
        })
    return in_maps


def kernel(x, qkv_w, qkv_b, proj_w, proj_b):
    x = np.asarray(x, dtype=np.float32)
    qkv_w = np.asarray(qkv_w, dtype=np.float32)
    qkv_b = np.asarray(qkv_b, dtype=np.float32)
    proj_w = np.asarray(proj_w, dtype=np.float32)
    proj_b = np.asarray(proj_b, dtype=np.float32)

    if "nc" not in _CACHE:
        _CACHE["nc"] = build_nc(reps=1)
    nc = _CACHE["nc"]

    in_maps = _prepare_inputs(x, qkv_w, qkv_b, proj_w, proj_b)
    res = run_bass_kernel_spmd(nc, in_maps, list(range(NCORES)))

    # host-side bias fold: v-bias through proj + proj bias
    pb_total = (proj_b + qkv_b[2 * C:] @ proj_w.T).astype(np.float32)
    out = np.empty((B, N, C), dtype=np.float32)
    for b in range(B):
        out[b] = res.results[G * b]["out"].astype(np.float32)
        out[b] += res.results[G * b + 1]["out"].astype(np.float32)
        out[b] += pb_total
    return out


# revision 4
# speedup vs baseline: 2.4265x; 1.0036x over previous
"""Multi-head attention on 8 Trainium2 NeuronCores — fp8-DoubleRow edition.

Problem: x[4, 2048, 768] -> qkv (12 heads, d=64) -> softmax attention -> proj.

Sharding: data-parallel over batch (4) x tensor-parallel over heads (2
groups of 6 heads) -> 8 shards; the host adds the two partial projections
per batch plus the (qkv v-bias + proj bias) fold (pure numpy adds).

All heavy matmuls run as fp8e4m3 MatmulPerfMode.DoubleRow (0.5 cycles/row;
pairing two 128-deep contraction tiles per matmul halves the row count
again), taking per-core PE busy from ~228us (f32r baseline) to ~133us. The
bottleneck becomes the ACT/DVE elementwise wall: softmax exp of 25.2M
logits/core plus all PSUM evacuation (Pool/GPSIMD cannot access PSUM and
DMA cannot read PSUM, so exactly two engines can touch PSUM). The exp is
split ACT (exact table exp -> fp8, ln16 bias) / DVE (Schraudolph integer
exp: e4m3 bits = int8(A8*x + B8), ~3% sawtooth) roughly 34:30 per pair;
Pool handles the softmax normalization (broadcast + multiply, all SBUF).

Numerics (e4m3 denormals start at 2^-6, so every small tensor is quantized
at a power-of-2 scale, undone downstream for free): weights at 32x (undone
in the evac affine), q/k at 4x (undone in the exp argument), v at 16x with
the PV sums column holding 16.0 so normalization cancels everything.
Accuracy is bought with residual terms where it is cheap: q/k = (x8 + xr) @
w8 + x8 @ wr (host-prepared fp8 residual tensors), and V feeds PV as TWO
fp8 tensors v_hi + v_lo with two DoubleRow accumulation chains (~bf16
quality at fp8 speed). pt is fp8 at 16x. proj stays f32r (fp8 would cost
2.4e-2 rel err alone; 3-term fp8 proj is exactly cost-neutral vs f32r with
an odd k-tile count). Output DMA is fp16, upcast host-side.

Hardware gotchas found on the way: dual-fp8 LdWeights requires the pair
stride to be a multiple of 128B (V is stored in 128B-aligned per-head
slots); a PSUM start=True zeroes the WHOLE 2KB bank, so exactly one matmul
per bank carries start; the QK DoubleRow puts the 64-channel contraction in
pair0 and zeros in pair1 so no partition remap of q/k is ever needed.

Schedule: sweep1 (PE-bound, ~44us) computes q/k for all pairs + V; three
fillerless attention pairs (~45us each, exp-bound, 3 st psum bufs so QK
never sits on the exp critical path; the normalization chain is deferred 4
tiles so the DVE reciprocal never heads the queue); f32r projection tail.
Cost-model span 204.6us (baseline 254.1us); measured rel err 1.64e-2
(gate 2e-2).
"""

import sys

sys.path.insert(0, "/opt/trn_rl_repo")

import numpy as np
import ml_dtypes

import concourse.bass as bass
import concourse.mybir as mybir
import concourse.tile as tile
from concourse import bacc
from concourse.bass_utils import run_bass_kernel_spmd

B, N, C, H, D = 4, 2048, 768, 12, 64
NCORES = 8
G = 2                    # head-parallel groups
CG = C // G              # 384 channels per group (6 heads)
HG = H // G              # 6 heads per core
CTG = CG // 128          # 3 head-pair slices (2 heads each)
KT = C // 128            # 6 contraction tiles (input channels)
KP = KT // 2             # 3 contraction k-tile pairs
TT = N // 128            # 16 key token tiles
TP = TT // 2             # 8 key token tile pairs
VW = 65                  # per-head V width (64 channels + sums column)
CHUNK = 512              # token chunk for qkv sweeps
NCH = N // CHUNK         # 4
SCALE = float(D) ** -0.5

WS = 32.0                # weight fp8 scale
QS = 4.0                 # q/k fp8 scale
VS = 16.0                # v fp8 scale
PK = 16.0                # pt fp8 scale

f32 = mybir.dt.float32
f32r = mybir.dt.float32r
fp8 = mybir.dt.float8e4
f16 = mybir.dt.float16
i8 = mybir.dt.int8
E4 = ml_dtypes.float8_e4m3

A8 = 8.0 / np.log(2.0)
B8 = 8.0 * (7 + np.log2(PK)) - 0.458
ESC = SCALE / (QS * QS)  # st psum holds 16x the raw logits
AF = mybir.ActivationFunctionType
ALU = mybir.AluOpType
PM = mybir.MatmulPerfMode

# exp engine per (q5, key tile): 'a'=ACT exact exp, 'd'=DVE Schraudolph
EXP_PAT = ("adadadadadadadad", "adadadadadadadad",
           "adadadadadadadad", "adadadadadadadad")
PT_BUFS = 10
PIPE_AHEAD = 8

_CACHE = {}


def build_nc(reps: int = 1):
    nc = bacc.Bacc("TRN2", target_bir_lowering=False, debug=False,
                   num_devices=NCORES)
    x8T = nc.dram_tensor("x8T", [C, N], fp8, kind="ExternalInput")
    xrT = nc.dram_tensor("xrT", [C, N], fp8, kind="ExternalInput")
    wqk8 = nc.dram_tensor("wqk8", [C, 2 * CG], fp8, kind="ExternalInput")
    wqkr = nc.dram_tensor("wqkr", [C, 2 * CG], fp8, kind="ExternalInput")
    wv8 = nc.dram_tensor("wv8", [C, CG], fp8, kind="ExternalInput")
    wvr = nc.dram_tensor("wvr", [C, CG], fp8, kind="ExternalInput")
    bqkW = nc.dram_tensor("bqkW", [128, 2 * CTG], f32, kind="ExternalInput")
    bqkQ = nc.dram_tensor("bqkQ", [128, 2 * CTG], f32, kind="ExternalInput")
    pwT = nc.dram_tensor("pwT", [CG, C], f32r, kind="ExternalInput")
    out = nc.dram_tensor("out", [N, C], f16, kind="ExternalOutput")
    import os
    dbg = {}
    if os.environ.get("KV2_DEBUG") == "1":
        dbg["q8"] = nc.dram_tensor("dbg_q8", [128, 2 * N], mybir.dt.uint8,
                                   kind="ExternalOutput")
        dbg["k8"] = nc.dram_tensor("dbg_k8", [128, 2 * N], mybir.dt.uint8,
                                   kind="ExternalOutput")
        dbg["vh"] = nc.dram_tensor("dbg_vh", [128, 2 * HG * 128],
                                   mybir.dt.uint8, kind="ExternalOutput")
        dbg["vl"] = nc.dram_tensor("dbg_vl", [128, 2 * HG * 128],
                                   mybir.dt.uint8, kind="ExternalOutput")
        dbg["at"] = nc.dram_tensor("dbg_at", [128, N], f32,
                                   kind="ExternalOutput")
        dbg["pt"] = nc.dram_tensor("dbg_pt", [128, 2048], mybir.dt.uint8,
                                   kind="ExternalOutput")

    with tile.TileContext(nc) as tc:
        body(nc, tc, x8T, xrT, wqk8, wqkr, wv8, wvr, bqkW, bqkQ, pwT, out,
             reps, dbg)
    nc.compile()
    return nc


def body(nc, tc, x8T, xrT, wqk8, wqkr, wv8, wvr, bqkW, bqkQ, pwT, out, reps,
         dbg=None):
    import contextlib

    loop_ctx = tc.For_i(0, reps, 1) if reps > 1 else contextlib.nullcontext()
    with loop_ctx:
        with tc.tile_pool(name="persist", bufs=1) as persist:
            # q8/k8: [128 part = 2 heads x 64ch, pair, token]; pair1 = zeros
            q8 = [persist.tile([128, 2, N], fp8, name=f"q8_{j}", tag=f"q8_{j}")
                  for j in range(CTG)]
            k8 = [persist.tile([128, 2, N], fp8, name=f"k8_{j}", tag=f"k8_{j}")
                  for j in range(CTG)]
            # vP_hi/lo[i]: key-tile-pair-interleaved V, one 128B-aligned slot
            # per head (dual-fp8 LdWeights needs the pair stride to be a
            # multiple of 128B): [128 key, pair, head, 128] with v in 0:64,
            # the sums column at 64, 65:128 unused.
            vHi = [persist.tile([128, 2, HG, 128], fp8, name=f"vH{i}",
                                tag=f"vH{i}") for i in range(TP)]
            vLo = [persist.tile([128, 2, HG, 128], fp8, name=f"vL{i}",
                                tag=f"vL{i}") for i in range(TP)]
            attnT = [persist.tile([128, N], f32r, name=f"attnT{j}",
                                  tag=f"attnT{j}") for j in range(CTG)]
            bW_sb = persist.tile([128, 2 * CTG], f32, tag="bW")
            bQ_sb = persist.tile([128, 2 * CTG], f32, tag="bQ")
            lnk_sb = persist.tile([128, 1], f32, tag="lnk")

            x8_kpn = x8T.rearrange("(k p) n -> p k n", p=128)
            xr_kpn = xrT.rearrange("(k p) n -> p k n", p=128)
            w8_kpn = wqk8.rearrange("(k p) n -> p k n", p=128)
            wr_kpn = wqkr.rearrange("(k p) n -> p k n", p=128)
            wv8_kpn = wv8.rearrange("(k p) n -> p k n", p=128)
            wvr_kpn = wvr.rearrange("(k p) n -> p k n", p=128)

            nc.vector.memset(lnk_sb[:], float(np.log(PK)))
            for j in range(CTG):
                nc.gpsimd.memset(q8[j][:, 1, :], 0.0)
                nc.gpsimd.memset(k8[j][:, 1, :], 0.0)
            # sums columns: VS in v_hi, 0 in v_lo
            for i in range(TP):
                nc.gpsimd.memset(vHi[i][:, :, :, D:D + 1], VS)
                nc.gpsimd.memset(vLo[i][:, :, :, D:D + 1], 0.0)

            def qk_evac(dst, ps, bcol, engine):
                # dst = QS*(ps/WS + bias) ; bW = WS*bias, bQ = QS*bias
                if engine == "act":
                    nc.scalar.activation(dst, ps[:], AF.Identity,
                                         bias=bQ_sb[:, bcol:bcol + 1],
                                         scale=QS / WS)
                else:
                    nc.vector.tensor_scalar(out=dst, in0=ps[:],
                                            scalar1=bW_sb[:, bcol:bcol + 1],
                                            scalar2=QS / WS,
                                            op0=ALU.add, op1=ALU.mult)

            def qk_mms(ps, col0, xR, xrR):
                """18 paired-DR matmuls: (x8+xr)@w8 + x8@wr -> [128,CHUNK]."""
                mms = []
                for xa, wa in ((xR, wqk_sb), (xrR, wqk_sb), (xR, wqkr_sb)):
                    for i in range(KP):
                        for qh in range(CHUNK // 256):
                            mms.append((
                                wa[:, 2 * i:2 * i + 2, col0:col0 + 128],
                                xa[:, 2 * i:2 * i + 2,
                                   qh * 256:qh * 256 + 256],
                                ps[:, qh * 256:qh * 256 + 256]))
                return mms

            def run_mms(mms, nfirst, nlast):
                # PSUM start zeroes the WHOLE 2KB bank, so only the first
                # matmul of a bank may carry start=True; later regions of
                # the same bank accumulate onto the zeroed bank.
                for n, (wa, xa, pd) in enumerate(mms):
                    nc.tensor.matmul(pd, wa, xa, start=(n == 0),
                                     stop=(n >= len(mms) - nlast),
                                     perf_mode=PM.DoubleRow,
                                     skip_group_check=True)

            # ---------- sweep 1: V (all heads) + k/q for pair 0 ----------
            with (
                tc.tile_pool(name="x1f", bufs=2) as x1f,
                tc.tile_pool(name="xr1f", bufs=2) as xr1f,
                tc.tile_pool(name="qkps1", bufs=2, space="PSUM") as qkps1,
                tc.tile_pool(name="vps1", bufs=3, space="PSUM") as vps1,
            ):
                wqk_sb = persist.tile([128, KT, 2 * CG], fp8, tag="wqk")
                nc.sync.dma_start(out=wqk_sb[:], in_=w8_kpn[:, :, :])
                xf0 = x1f.tile([128, KT, CHUNK], fp8, tag="xf", name="xf_s1_0")
                nc.sync.dma_start(out=xf0[:], in_=x8_kpn[:, :, 0:CHUNK])
                xr0 = xr1f.tile([128, KT, CHUNK], fp8, tag="xr", name="xr_s1_0")
                nc.sync.dma_start(out=xr0[:], in_=xr_kpn[:, :, 0:CHUNK])
                wqkr_sb = persist.tile([128, KT, 2 * CG], fp8, tag="wqkr")
                nc.sync.dma_start(out=wqkr_sb[:], in_=wr_kpn[:, :, :])
                nc.sync.dma_start(out=bW_sb[:], in_=bqkW[:, :])
                nc.sync.dma_start(out=bQ_sb[:], in_=bqkQ[:, :])
                wv_sb = persist.tile([128, KT, CG], fp8, tag="wv8")
                nc.sync.dma_start(out=wv_sb[:], in_=wv8_kpn[:, :, :])
                wr_sb = persist.tile([128, KT, CG], fp8, tag="wvr")
                nc.sync.dma_start(out=wr_sb[:], in_=wvr_kpn[:, :, :])

                xR, xrR = xf0, xr0
                for u in range(NCH):
                    lo = u * CHUNK
                    for j in range(CTG):
                        psk = qkps1.tile([128, CHUNK], f32, tag="qk",
                                         name=f"psk{u}_{j}")
                        run_mms(qk_mms(psk, CG + j * 128, xR, xrR), 2, 2)
                        qk_evac(k8[j][:, 0, lo:lo + CHUNK], psk, CTG + j,
                                "act")
                        psq = qkps1.tile([128, CHUNK], f32, tag="qk",
                                         name=f"psq{u}_{j}")
                        run_mms(qk_mms(psq, j * 128, xR, xrR), 2, 2)
                        qk_evac(q8[j][:, 0, lo:lo + CHUNK], psq, j, "vector")
                    if u + 1 < NCH:
                        nlo = lo + CHUNK
                        xR_next = x1f.tile([128, KT, CHUNK], fp8, tag="xf",
                                           name=f"xf_s1_{u + 1}")
                        nc.sync.dma_start(out=xR_next[:],
                                          in_=x8_kpn[:, :, nlo:nlo + CHUNK])
                        xr_next = xr1f.tile([128, KT, CHUNK], fp8, tag="xr",
                                            name=f"xr_s1_{u + 1}")
                        nc.sync.dma_start(out=xr_next[:],
                                          in_=xr_kpn[:, :, nlo:nlo + CHUNK])
                    else:
                        xR_next = xr_next = None
                    # V for this chunk's 4 token tiles: 3-term fp8
                    for tloc in range(CHUNK // 128):
                        t = u * (CHUNK // 128) + tloc
                        tsl = slice(tloc * 128, (tloc + 1) * 128)
                        ps = vps1.tile([128, CG], f32, tag="v",
                                       name=f"psv{t}")
                        mms = []
                        for xa, wa in ((xR, wv_sb), (xrR, wv_sb),
                                       (xR, wr_sb)):
                            for i in range(KP):
                                for hf in range(CG // 128):
                                    csl = slice(hf * 128, hf * 128 + 128)
                                    mms.append((xa[:, 2 * i:2 * i + 2, tsl],
                                                wa[:, 2 * i:2 * i + 2, csl],
                                                ps[:, csl]))
                        run_mms(mms, CG // 128, CG // 128)
                        hvh = vHi[t // 2]
                        hvl = vLo[t // 2]
                        psh = ps[:].rearrange("p (h d) -> p h d", d=D)
                        # v_hi = VS/WS * ps ; v_lo = VS/WS * ps - v_hi
                        nc.scalar.activation(hvh[:, t % 2, :, 0:D], psh,
                                             AF.Identity, scale=VS / WS)
                        nc.vector.scalar_tensor_tensor(
                            out=hvl[:, t % 2, :, 0:D], in0=psh,
                            scalar=VS / WS, in1=hvh[:, t % 2, :, 0:D],
                            op0=ALU.mult, op1=ALU.subtract)
                    xR, xrR = xR_next, xr_next

            # ---------- attention + interleaved QKV slices + proj ----------
            with (
                tc.tile_pool(name="ptpool", bufs=PT_BUFS) as ptpool,
                tc.tile_pool(name="rlpool", bufs=2) as rlpool,
                tc.tile_pool(name="bcpool", bufs=2) as bcpool,
                tc.tile_pool(name="stps", bufs=3, space="PSUM") as stps,
                tc.tile_pool(name="otps", bufs=1, space="PSUM") as otps,
            ):
                def pull(filler, n):
                    for _ in range(n):
                        if filler is None:
                            return
                        try:
                            next(filler)
                        except StopIteration:
                            return

                it_state = {"it": 0}

                def attention_pair(j, filler=None, budget=None):
                    if budget is None:
                        budget = lambda it: 3
                    NQ5 = N // 512
                    seq = [(q5, t) for q5 in range(NQ5) for t in range(TT)]
                    ots = {}

                    def st_exp(q5, t):
                        qlo = q5 * 512
                        st = stps.tile([128, 1024], f32, tag="st",
                                       name=f"st_{j}_{q5}_{t}")
                        for h in range(2):
                            hp = slice(64 * h, 64 * h + 64)
                            for qh in range(2):
                                qsl = slice(qlo + qh * 256,
                                            qlo + qh * 256 + 256)
                                nc.tensor.matmul(
                                    st[:, h * 512 + qh * 256:
                                       h * 512 + qh * 256 + 256],
                                    k8[j][hp, :, t * 128:(t + 1) * 128],
                                    q8[j][hp, :, qsl],
                                    start=(qh == 0), stop=True,
                                    perf_mode=PM.DoubleRow,
                                    skip_group_check=True)
                        return st

                    def emit_exp(q5, t, st, ptP):
                        if EXP_PAT[q5][t] == "a":
                            nc.scalar.activation(
                                ptP.bitcast(fp8)[:, t % 2, :], st[:], AF.Exp,
                                bias=lnk_sb[:, 0:1], scale=ESC)
                        else:
                            nc.vector.tensor_scalar(
                                out=ptP[:, t % 2, :], in0=st[:],
                                scalar1=A8 * ESC, scalar2=B8,
                                op0=ALU.mult, op1=ALU.add)

                    def emit_pv(q5, i, ptP):
                        ot = ots[q5]
                        pt8 = ptP.bitcast(fp8).rearrange(
                            "p two (h q) -> p two h q", h=2)
                        for h in range(2):
                            for qh in range(2):
                                osl = slice(h * 512 + qh * 256,
                                            h * 512 + qh * 256 + 256)
                                rhs = pt8[:, :, h, qh * 256:qh * 256 + 256]
                                nc.tensor.matmul(
                                    ot[:, osl],
                                    vHi[i][:, :, 2 * j + h, 0:VW], rhs,
                                    start=(i == 0 and qh == 0), stop=False,
                                    perf_mode=PM.DoubleRow,
                                    skip_group_check=True)
                                nc.tensor.matmul(
                                    ot[:, osl],
                                    vLo[i][:, :, 2 * j + h, 0:VW], rhs,
                                    start=False, stop=(i == TP - 1),
                                    perf_mode=PM.DoubleRow,
                                    skip_group_check=True)

                    osbs = {}

                    def emit_evac_copy(q5):
                        ot = ots.pop(q5)
                        # high-priority copy frees the psum banks fast
                        osb = rlpool.tile([VW, 1024], f32, tag="osb",
                                          name=f"osb_{j}_{q5}")
                        with tc.high_priority():
                            nc.scalar.activation(osb[:], ot[:], AF.Copy)
                        osbs[q5] = osb

                    def emit_evac_norm(q5):
                        qlo = q5 * 512
                        osb = osbs.pop(q5)
                        rl = rlpool.tile([1, 1024], f32, tag="rl",
                                         name=f"rl_{j}_{q5}")
                        nc.vector.reciprocal(rl[0:1, :], osb[D:D + 1, :])
                        bc = bcpool.tile([64, 1024], f32, tag="bc",
                                         name=f"bc_{j}_{q5}")
                        nc.gpsimd.partition_broadcast(bc[:], rl[0:1, :])
                        for h in range(2):
                            nc.gpsimd.tensor_tensor(
                                out=attnT[j][64 * h:64 * h + 64,
                                             qlo:qlo + 512],
                                in0=osb[0:D, h * 512:h * 512 + 512],
                                in1=bc[:, h * 512:h * 512 + 512],
                                op=ALU.mult)

                    def new_pt(q5, i):
                        return ptpool.tile([128, 2, 1024], i8, tag="pt",
                                           name=f"pt_{j}_{q5}_{i}")

                    pts = {}
                    pending = []
                    for idx in range(PIPE_AHEAD):
                        q5, t = seq[idx]
                        if t % 2 == 0:
                            pts[(q5, t // 2)] = new_pt(q5, t // 2)
                        emit_exp(q5, t, st_exp(q5, t), pts[(q5, t // 2)])
                    for it, (q5, t) in enumerate(seq):
                        if t == 0:
                            ots[q5] = otps.tile([VW, 1024], f32, tag="ot",
                                                name=f"ot_{j}_{q5}")
                        # exp pipeline first so PE/engine queues never sit
                        # behind a blocked PV or a waiting recip
                        if it + PIPE_AHEAD < len(seq):
                            q5n, tn = seq[it + PIPE_AHEAD]
                            if tn % 2 == 0:
                                pts[(q5n, tn // 2)] = new_pt(q5n, tn // 2)
                            emit_exp(q5n, tn, st_exp(q5n, tn),
                                     pts[(q5n, tn // 2)])
                        if t % 2 == 1:
                            emit_pv(q5, t // 2, pts.pop((q5, t // 2)))
                        if t == TT - 1:
                            emit_evac_copy(q5)
                            pending.append((q5, t))
                        # deferred: the recip/bcast/mults of q5 run 4 tiles
                        # into q5+1 so the recip never heads the DVE queue
                        if pending and (t == 4 or it == len(seq) - 1):
                            emit_evac_norm(pending.pop(0)[0])
                        it_state["it"] = it
                        pull(filler, budget(it))
                    while pending:
                        emit_evac_norm(pending.pop(0)[0])

                import os
                phases = os.environ.get("KV2_PHASES", "s1,p0,p1,p2,proj").split(",")
                for j in range(CTG - 1):
                    if f"p{j}" in phases:
                        attention_pair(j)

                if "p2" in phases:
                    attention_pair(CTG - 1)

            # ------------------- projection tail phase -------------------
            with (
                tc.tile_pool(name="outsb", bufs=3) as outsb,
                tc.tile_pool(name="prps", bufs=2, space="PSUM") as prps,
            ):
                if True:
                    pwT_kpn = pwT.rearrange("(k p) n -> p k n", p=128)
                    pwf = persist.tile([128, CTG, C], f32r, tag="pwf")
                    nc.sync.dma_start(out=pwf[:], in_=pwT_kpn[:, :, :])

                    def proj_tok_tile(tt):
                        osb = outsb.tile([128, C], f16, tag="osb",
                                         name=f"osb_p{tt}")
                        for half in range(2):
                            ps = prps.tile([128, C // 2], f32, tag="pr",
                                           name=f"prps_{tt}_{half}")
                            for k in range(CTG):
                                nc.tensor.matmul(
                                    ps[:],
                                    attnT[k][:, tt * 128:(tt + 1) * 128],
                                    pwf[:, k, half * (C // 2):
                                        (half + 1) * (C // 2)],
                                    start=(k == 0), stop=(k == CTG - 1))
                                yield
                            dst = osb[:, half * (C // 2):(half + 1) * (C // 2)]
                            if half == 0:
                                nc.scalar.activation(dst, ps[:], AF.Copy)
                            else:
                                nc.vector.tensor_copy(dst, ps[:])
                        nc.sync.dma_start(
                            out=out[tt * 128:(tt + 1) * 128, :], in_=osb[:])

                    rest = range(TT) if "proj" in phases else []
                    for tt in rest:
                        for _ in proj_tok_tile(tt):
                            pass
                    if dbg:
                        nc.sync.dma_start(out=dbg["q8"][:, :],
                                          in_=q8[0].bitcast(mybir.dt.uint8)[:, :, :])
                        nc.sync.dma_start(out=dbg["k8"][:, :],
                                          in_=k8[0].bitcast(mybir.dt.uint8)[:, :, :])
                        nc.sync.dma_start(out=dbg["vh"][:, :],
                                          in_=vHi[0].bitcast(mybir.dt.uint8)[:, :, :, :])
                        nc.sync.dma_start(out=dbg["vl"][:, :],
                                          in_=vLo[0].bitcast(mybir.dt.uint8)[:, :, :, :])
                        nc.sync.dma_start(out=dbg["at"][:, :],
                                          in_=attnT[0].bitcast(f32)[:, :])


def _prepare_inputs(x, qkv_w, qkv_b, proj_w, proj_b):
    """Host-side shard preparation (numpy quantize/reshape/transpose)."""
    x = np.asarray(x, dtype=np.float32)
    x8 = x.astype(E4)
    xr = (x - x8.astype(np.float32)).astype(E4)
    in_maps = []
    for core in range(NCORES):
        b, s = core // G, core % G
        rq = slice(s * CG, (s + 1) * CG)
        rk = slice(C + s * CG, C + (s + 1) * CG)
        rv = slice(2 * C + s * CG, 2 * C + (s + 1) * CG)
        wqk = np.concatenate([qkv_w[rq], qkv_w[rk]], 0).T  # [768, 768]
        wqk8 = (WS * wqk).astype(E4)
        wqkr = (WS * wqk - wqk8.astype(np.float32)).astype(E4)
        wv = qkv_w[rv].T  # [768, 384]
        wv8 = (WS * wv).astype(E4)
        wvr = (WS * wv - wv8.astype(np.float32)).astype(E4)
        bvec = np.concatenate([qkv_b[rq], qkv_b[rk]])
        bmat = bvec.reshape(2 * CTG, 128).T
        pwT = np.ascontiguousarray(proj_w.T[s * CG:(s + 1) * CG, :]).astype(
            np.float32)
        in_maps.append({
            "x8T": np.ascontiguousarray(x8[b].T),
            "xrT": np.ascontiguousarray(xr[b].T),
            "wqk8": np.ascontiguousarray(wqk8),
            "wqkr": np.ascontiguousarray(wqkr),
            "wv8": np.ascontiguousarray(wv8),
            "wvr": np.ascontiguousarray(wvr),
            "bqkW": np.ascontiguousarray(WS * bmat).astype(np.float32),
            "bqkQ": np.ascontiguousarray(QS * bmat).astype(np.float32),
            "pwT": pwT,
        })
    return in_maps


def kernel(x, qkv_w, qkv_b, proj_w, proj_b):
    x = np.asarray(x, dtype=np.float32)
    qkv_w = np.asarray(qkv_w, dtype=np.float32)
    qkv_b = np.asarray(qkv_b, dtype=np.float32)
    proj_w = np.asarray(proj_w, dtype=np.float32)
    proj_b = np.asarray(proj_b, dtype=np.float32)

    if "nc" not in _CACHE:
        _CACHE["nc"] = build_nc(reps=1)
    nc = _CACHE["nc"]

    in_maps = _prepare_inputs(x, qkv_w, qkv_b, proj_w, proj_b)
    res = run_bass_kernel_spmd(nc, in_maps, list(range(NCORES)))

    # host-side bias fold: v-bias through proj + proj bias
    pb_total = (proj_b + qkv_b[2 * C:] @ proj_w.T).astype(np.float32)
    out = np.empty((B, N, C), dtype=np.float32)
    for b in range(B):
        out[b] = res.results[G * b]["out"].astype(np.float32)
        out[b] += res.results[G * b + 1]["out"].astype(np.float32)
        out[b] += pb_total
    return out


# revision 5
# speedup vs baseline: 2.4471x; 1.0085x over previous
"""Multi-head attention on 8 Trainium2 NeuronCores — fp8-DoubleRow edition.

Problem: x[4, 2048, 768] -> qkv (12 heads, d=64) -> softmax attention -> proj.

Sharding: data-parallel over batch (4) x tensor-parallel over heads (2
groups of 6 heads) -> 8 shards; the host adds the two partial projections
per batch plus the (qkv v-bias + proj bias) fold (pure numpy adds).

All heavy matmuls run as fp8e4m3 MatmulPerfMode.DoubleRow (0.5 cycles/row;
pairing two 128-deep contraction tiles per matmul halves the row count
again), taking per-core PE busy from ~228us (f32r baseline) to ~133us. The
bottleneck becomes the ACT/DVE elementwise wall: softmax exp of 25.2M
logits/core plus all PSUM evacuation (Pool/GPSIMD cannot access PSUM and
DMA cannot read PSUM, so exactly two engines can touch PSUM). The exp is
split ACT (exact table exp -> fp8, ln16 bias) / DVE (Schraudolph integer
exp: e4m3 bits = int8(A8*x + B8), ~3% sawtooth) roughly 34:30 per pair;
Pool handles the softmax normalization (broadcast + multiply, all SBUF).

Numerics (e4m3 denormals start at 2^-6, so every small tensor is quantized
at a power-of-2 scale, undone downstream for free): weights at 32x (undone
in the evac affine), q/k at 4x (undone in the exp argument), v at 16x with
the PV sums column holding 16.0 so normalization cancels everything.
Accuracy is bought with residual terms where it is cheap: q/k = (x8 + xr) @
w8 + x8 @ wr (host-prepared fp8 residual tensors), and V feeds PV as TWO
fp8 tensors v_hi + v_lo with two DoubleRow accumulation chains (~bf16
quality at fp8 speed). pt is fp8 at 16x. proj stays f32r (fp8 would cost
2.4e-2 rel err alone; 3-term fp8 proj is exactly cost-neutral vs f32r with
an odd k-tile count). Output DMA is fp16, upcast host-side.

Hardware gotchas found on the way: dual-fp8 LdWeights requires the pair
stride to be a multiple of 128B (V is stored in 128B-aligned per-head
slots); a PSUM start=True zeroes the WHOLE 2KB bank, so exactly one matmul
per bank carries start; the QK DoubleRow puts the 64-channel contraction in
pair0 and zeros in pair1 so no partition remap of q/k is ever needed.

Schedule: sweep1 (PE-bound, ~44us) computes q/k for all pairs + V; three
fillerless attention pairs (~45us each, exp-bound, 3 st psum bufs so QK
never sits on the exp critical path; the normalization chain is deferred 4
tiles so the DVE reciprocal never heads the queue); f32r projection tail.
Cost-model span 204.6us (baseline 254.1us); measured rel err 1.64e-2
(gate 2e-2).
"""

import sys

sys.path.insert(0, "/opt/trn_rl_repo")

import numpy as np
import ml_dtypes

import concourse.bass as bass
import concourse.mybir as mybir
import concourse.tile as tile
from concourse import bacc
from concourse.bass_utils import run_bass_kernel_spmd

B, N, C, H, D = 4, 2048, 768, 12, 64
NCORES = 8
G = 2                    # head-parallel groups
CG = C // G              # 384 channels per group (6 heads)
HG = H // G              # 6 heads per core
CTG = CG // 128          # 3 head-pair slices (2 heads each)
KT = C // 128            # 6 contraction tiles (input channels)
KP = KT // 2             # 3 contraction k-tile pairs
TT = N // 128            # 16 key token tiles
TP = TT // 2             # 8 key token tile pairs
VW = 65                  # per-head V width (64 channels + sums column)
CHUNK = 512              # token chunk for qkv sweeps
NCH = N // CHUNK         # 4
SCALE = float(D) ** -0.5

WS = 32.0                # weight fp8 scale
QS = 4.0                 # q/k fp8 scale
VS = 16.0                # v fp8 scale
PK = 16.0                # pt fp8 scale

f32 = mybir.dt.float32
f32r = mybir.dt.float32r
fp8 = mybir.dt.float8e4
f16 = mybir.dt.float16
i8 = mybir.dt.int8
E4 = ml_dtypes.float8_e4m3

A8 = 8.0 / np.log(2.0)
B8 = 8.0 * (7 + np.log2(PK)) - 0.458
ESC = SCALE / (QS * QS)  # st psum holds 16x the raw logits
AF = mybir.ActivationFunctionType
ALU = mybir.AluOpType
PM = mybir.MatmulPerfMode

# exp engine per (q5, key tile): 'a'=ACT exact exp, 'd'=DVE Schraudolph
EXP_PAT = ("adadadadadadadad", "adadadadadadadad",
           "adadadadadadadad", "adadadadadadadad")
PT_BUFS = 10
PIPE_AHEAD = 8

_CACHE = {}


def build_nc(reps: int = 1):
    nc = bacc.Bacc("TRN2", target_bir_lowering=False, debug=False,
                   num_devices=NCORES)
    x8T = nc.dram_tensor("x8T", [C, N], fp8, kind="ExternalInput")
    xrT = nc.dram_tensor("xrT", [C, N], fp8, kind="ExternalInput")
    wqk8 = nc.dram_tensor("wqk8", [C, 2 * CG], fp8, kind="ExternalInput")
    wqkr = nc.dram_tensor("wqkr", [C, 2 * CG], fp8, kind="ExternalInput")
    wv8 = nc.dram_tensor("wv8", [C, CG], fp8, kind="ExternalInput")
    wvr = nc.dram_tensor("wvr", [C, CG], fp8, kind="ExternalInput")
    bqkW = nc.dram_tensor("bqkW", [128, 2 * CTG], f32, kind="ExternalInput")
    bqkQ = nc.dram_tensor("bqkQ", [128, 2 * CTG], f32, kind="ExternalInput")
    pwT = nc.dram_tensor("pwT", [CG, C], f32r, kind="ExternalInput")
    out = nc.dram_tensor("out", [N, C], f16, kind="ExternalOutput")
    import os
    dbg = {}
    if os.environ.get("KV2_DEBUG") == "1":
        dbg["q8"] = nc.dram_tensor("dbg_q8", [128, 2 * N], mybir.dt.uint8,
                                   kind="ExternalOutput")
        dbg["k8"] = nc.dram_tensor("dbg_k8", [128, 2 * N], mybir.dt.uint8,
                                   kind="ExternalOutput")
        dbg["vh"] = nc.dram_tensor("dbg_vh", [128, 2 * HG * 128],
                                   mybir.dt.uint8, kind="ExternalOutput")
        dbg["vl"] = nc.dram_tensor("dbg_vl", [128, 2 * HG * 128],
                                   mybir.dt.uint8, kind="ExternalOutput")
        dbg["at"] = nc.dram_tensor("dbg_at", [128, N], f32,
                                   kind="ExternalOutput")
        dbg["pt"] = nc.dram_tensor("dbg_pt", [128, 2048], mybir.dt.uint8,
                                   kind="ExternalOutput")

    with tile.TileContext(nc) as tc:
        body(nc, tc, x8T, xrT, wqk8, wqkr, wv8, wvr, bqkW, bqkQ, pwT, out,
             reps, dbg)
    nc.compile()
    return nc


def body(nc, tc, x8T, xrT, wqk8, wqkr, wv8, wvr, bqkW, bqkQ, pwT, out, reps,
         dbg=None):
    import contextlib

    loop_ctx = tc.For_i(0, reps, 1) if reps > 1 else contextlib.nullcontext()
    with loop_ctx:
        with tc.tile_pool(name="persist", bufs=1) as persist:
            # q8/k8: [128 part = 2 heads x 64ch, pair, token]; pair1 = zeros
            q8 = [persist.tile([128, 2, N], fp8, name=f"q8_{j}", tag=f"q8_{j}")
                  for j in range(CTG)]
            k8 = [persist.tile([128, 2, N], fp8, name=f"k8_{j}", tag=f"k8_{j}")
                  for j in range(CTG)]
            # vP_hi/lo[i]: key-tile-pair-interleaved V, one 128B-aligned slot
            # per head (dual-fp8 LdWeights needs the pair stride to be a
            # multiple of 128B): [128 key, pair, head, 128] with v in 0:64,
            # the sums column at 64, 65:128 unused.
            vHi = [persist.tile([128, 2, HG, 128], fp8, name=f"vH{i}",
                                tag=f"vH{i}") for i in range(TP)]
            vLo = [persist.tile([128, 2, HG, 128], fp8, name=f"vL{i}",
                                tag=f"vL{i}") for i in range(TP)]
            attnT = [persist.tile([128, N], f32r, name=f"attnT{j}",
                                  tag=f"attnT{j}") for j in range(CTG)]
            bW_sb = persist.tile([128, 2 * CTG], f32, tag="bW")
            bQ_sb = persist.tile([128, 2 * CTG], f32, tag="bQ")
            lnk_sb = persist.tile([128, 1], f32, tag="lnk")

            x8_kpn = x8T.rearrange("(k p) n -> p k n", p=128)
            xr_kpn = xrT.rearrange("(k p) n -> p k n", p=128)
            w8_kpn = wqk8.rearrange("(k p) n -> p k n", p=128)
            wr_kpn = wqkr.rearrange("(k p) n -> p k n", p=128)
            wv8_kpn = wv8.rearrange("(k p) n -> p k n", p=128)
            wvr_kpn = wvr.rearrange("(k p) n -> p k n", p=128)

            nc.vector.memset(lnk_sb[:], float(np.log(PK)))
            for j in range(CTG):
                nc.gpsimd.memset(q8[j][:, 1, :], 0.0)
                nc.gpsimd.memset(k8[j][:, 1, :], 0.0)
            # sums columns: VS in v_hi, 0 in v_lo
            for i in range(TP):
                nc.gpsimd.memset(vHi[i][:, :, :, D:D + 1], VS)
                nc.gpsimd.memset(vLo[i][:, :, :, D:D + 1], 0.0)

            def qk_evac(dst, ps, bcol, engine):
                # dst = QS*(ps/WS + bias) ; bW = WS*bias, bQ = QS*bias
                if engine == "act":
                    nc.scalar.activation(dst, ps[:], AF.Identity,
                                         bias=bQ_sb[:, bcol:bcol + 1],
                                         scale=QS / WS)
                else:
                    nc.vector.tensor_scalar(out=dst, in0=ps[:],
                                            scalar1=bW_sb[:, bcol:bcol + 1],
                                            scalar2=QS / WS,
                                            op0=ALU.add, op1=ALU.mult)

            def qk_mms(ps, col0, xR, xrR):
                """18 paired-DR matmuls: (x8+xr)@w8 + x8@wr -> [128,CHUNK]."""
                mms = []
                for xa, wa in ((xR, wqk_sb), (xrR, wqk_sb), (xR, wqkr_sb)):
                    for i in range(KP):
                        for qh in range(CHUNK // 256):
                            mms.append((
                                wa[:, 2 * i:2 * i + 2, col0:col0 + 128],
                                xa[:, 2 * i:2 * i + 2,
                                   qh * 256:qh * 256 + 256],
                                ps[:, qh * 256:qh * 256 + 256]))
                return mms

            def run_mms(mms, nfirst, nlast):
                # PSUM start zeroes the WHOLE 2KB bank, so only the first
                # matmul of a bank may carry start=True; later regions of
                # the same bank accumulate onto the zeroed bank.
                for n, (wa, xa, pd) in enumerate(mms):
                    nc.tensor.matmul(pd, wa, xa, start=(n == 0),
                                     stop=(n >= len(mms) - nlast),
                                     perf_mode=PM.DoubleRow,
                                     skip_group_check=True)

            # ---------- sweep 1: V (all heads) + k/q for pair 0 ----------
            with (
                tc.tile_pool(name="x1f", bufs=2) as x1f,
                tc.tile_pool(name="xr1f", bufs=2) as xr1f,
                tc.tile_pool(name="qkps1", bufs=3, space="PSUM") as qkps1,
                tc.tile_pool(name="vps1", bufs=4, space="PSUM") as vps1,
            ):
                wqk_sb = persist.tile([128, KT, 2 * CG], fp8, tag="wqk")
                nc.sync.dma_start(out=wqk_sb[:], in_=w8_kpn[:, :, :])
                xf0 = x1f.tile([128, KT, CHUNK], fp8, tag="xf", name="xf_s1_0")
                nc.sync.dma_start(out=xf0[:], in_=x8_kpn[:, :, 0:CHUNK])
                xr0 = xr1f.tile([128, KT, CHUNK], fp8, tag="xr", name="xr_s1_0")
                nc.sync.dma_start(out=xr0[:], in_=xr_kpn[:, :, 0:CHUNK])
                wqkr_sb = persist.tile([128, KT, 2 * CG], fp8, tag="wqkr")
                nc.sync.dma_start(out=wqkr_sb[:], in_=wr_kpn[:, :, :])
                nc.sync.dma_start(out=bW_sb[:], in_=bqkW[:, :])
                nc.sync.dma_start(out=bQ_sb[:], in_=bqkQ[:, :])
                wv_sb = persist.tile([128, KT, CG], fp8, tag="wv8")
                nc.sync.dma_start(out=wv_sb[:], in_=wv8_kpn[:, :, :])
                wr_sb = persist.tile([128, KT, CG], fp8, tag="wvr")
                nc.sync.dma_start(out=wr_sb[:], in_=wvr_kpn[:, :, :])

                xR, xrR = xf0, xr0
                for u in range(NCH):
                    lo = u * CHUNK
                    for j in range(CTG):
                        psk = qkps1.tile([128, CHUNK], f32, tag="qk",
                                         name=f"psk{u}_{j}")
                        run_mms(qk_mms(psk, CG + j * 128, xR, xrR), 2, 2)
                        qk_evac(k8[j][:, 0, lo:lo + CHUNK], psk, CTG + j,
                                "act")
                        psq = qkps1.tile([128, CHUNK], f32, tag="qk",
                                         name=f"psq{u}_{j}")
                        run_mms(qk_mms(psq, j * 128, xR, xrR), 2, 2)
                        qk_evac(q8[j][:, 0, lo:lo + CHUNK], psq, j, "vector")
                    if u + 1 < NCH:
                        nlo = lo + CHUNK
                        xR_next = x1f.tile([128, KT, CHUNK], fp8, tag="xf",
                                           name=f"xf_s1_{u + 1}")
                        nc.sync.dma_start(out=xR_next[:],
                                          in_=x8_kpn[:, :, nlo:nlo + CHUNK])
                        xr_next = xr1f.tile([128, KT, CHUNK], fp8, tag="xr",
                                            name=f"xr_s1_{u + 1}")
                        nc.sync.dma_start(out=xr_next[:],
                                          in_=xr_kpn[:, :, nlo:nlo + CHUNK])
                    else:
                        xR_next = xr_next = None
                    # V for this chunk's 4 token tiles: 3-term fp8
                    for tloc in range(CHUNK // 128):
                        t = u * (CHUNK // 128) + tloc
                        tsl = slice(tloc * 128, (tloc + 1) * 128)
                        ps = vps1.tile([128, CG], f32, tag="v",
                                       name=f"psv{t}")
                        mms = []
                        for xa, wa in ((xR, wv_sb), (xrR, wv_sb),
                                       (xR, wr_sb)):
                            for i in range(KP):
                                for hf in range(CG // 128):
                                    csl = slice(hf * 128, hf * 128 + 128)
                                    mms.append((xa[:, 2 * i:2 * i + 2, tsl],
                                                wa[:, 2 * i:2 * i + 2, csl],
                                                ps[:, csl]))
                        run_mms(mms, CG // 128, CG // 128)
                        hvh = vHi[t // 2]
                        hvl = vLo[t // 2]
                        psh = ps[:].rearrange("p (h d) -> p h d", d=D)
                        # v_hi = VS/WS * ps ; v_lo = VS/WS * ps - v_hi
                        nc.scalar.activation(hvh[:, t % 2, :, 0:D], psh,
                                             AF.Identity, scale=VS / WS)
                        nc.vector.scalar_tensor_tensor(
                            out=hvl[:, t % 2, :, 0:D], in0=psh,
                            scalar=VS / WS, in1=hvh[:, t % 2, :, 0:D],
                            op0=ALU.mult, op1=ALU.subtract)
                    xR, xrR = xR_next, xr_next

            # ---------- attention + interleaved QKV slices + proj ----------
            with (
                tc.tile_pool(name="ptpool", bufs=PT_BUFS) as ptpool,
                tc.tile_pool(name="rlpool", bufs=2) as rlpool,
                tc.tile_pool(name="bcpool", bufs=2) as bcpool,
                tc.tile_pool(name="stps", bufs=3, space="PSUM") as stps,
                tc.tile_pool(name="otps", bufs=1, space="PSUM") as otps,
            ):
                def pull(filler, n):
                    for _ in range(n):
                        if filler is None:
                            return
                        try:
                            next(filler)
                        except StopIteration:
                            return

                it_state = {"it": 0}

                def attention_pair(j, filler=None, budget=None):
                    if budget is None:
                        budget = lambda it: 3
                    NQ5 = N // 512
                    seq = [(q5, t) for q5 in range(NQ5) for t in range(TT)]
                    ots = {}

                    def st_exp(q5, t):
                        qlo = q5 * 512
                        st = stps.tile([128, 1024], f32, tag="st",
                                       name=f"st_{j}_{q5}_{t}")
                        for h in range(2):
                            hp = slice(64 * h, 64 * h + 64)
                            for qh in range(2):
                                qsl = slice(qlo + qh * 256,
                                            qlo + qh * 256 + 256)
                                nc.tensor.matmul(
                                    st[:, h * 512 + qh * 256:
                                       h * 512 + qh * 256 + 256],
                                    k8[j][hp, :, t * 128:(t + 1) * 128],
                                    q8[j][hp, :, qsl],
                                    start=(qh == 0), stop=True,
                                    perf_mode=PM.DoubleRow,
                                    skip_group_check=True)
                        return st

                    def emit_exp(q5, t, st, ptP):
                        if EXP_PAT[q5][t] == "a":
                            nc.scalar.activation(
                                ptP.bitcast(fp8)[:, t % 2, :], st[:], AF.Exp,
                                bias=lnk_sb[:, 0:1], scale=ESC)
                        else:
                            nc.vector.tensor_scalar(
                                out=ptP[:, t % 2, :], in0=st[:],
                                scalar1=A8 * ESC, scalar2=B8,
                                op0=ALU.mult, op1=ALU.add)

                    def emit_pv(q5, i, ptP):
                        ot = ots[q5]
                        pt8 = ptP.bitcast(fp8).rearrange(
                            "p two (h q) -> p two h q", h=2)
                        for h in range(2):
                            for qh in range(2):
                                osl = slice(h * 512 + qh * 256,
                                            h * 512 + qh * 256 + 256)
                                rhs = pt8[:, :, h, qh * 256:qh * 256 + 256]
                                nc.tensor.matmul(
                                    ot[:, osl],
                                    vHi[i][:, :, 2 * j + h, 0:VW], rhs,
                                    start=(i == 0 and qh == 0), stop=False,
                                    perf_mode=PM.DoubleRow,
                                    skip_group_check=True)
                                nc.tensor.matmul(
                                    ot[:, osl],
                                    vLo[i][:, :, 2 * j + h, 0:VW], rhs,
                                    start=False, stop=(i == TP - 1),
                                    perf_mode=PM.DoubleRow,
                                    skip_group_check=True)

                    osbs = {}

                    def emit_evac_copy(q5):
                        ot = ots.pop(q5)
                        # high-priority copy frees the psum banks fast
                        osb = rlpool.tile([VW, 1024], f32, tag="osb",
                                          name=f"osb_{j}_{q5}")
                        with tc.high_priority():
                            nc.scalar.activation(osb[:], ot[:], AF.Copy)
                        osbs[q5] = osb

                    def emit_evac_norm(q5):
                        qlo = q5 * 512
                        osb = osbs.pop(q5)
                        rl = rlpool.tile([1, 1024], f32, tag="rl",
                                         name=f"rl_{j}_{q5}")
                        nc.vector.reciprocal(rl[0:1, :], osb[D:D + 1, :])
                        bc = bcpool.tile([64, 1024], f32, tag="bc",
                                         name=f"bc_{j}_{q5}")
                        nc.gpsimd.partition_broadcast(bc[:], rl[0:1, :])
                        for h in range(2):
                            nc.gpsimd.tensor_tensor(
                                out=attnT[j][64 * h:64 * h + 64,
                                             qlo:qlo + 512],
                                in0=osb[0:D, h * 512:h * 512 + 512],
                                in1=bc[:, h * 512:h * 512 + 512],
                                op=ALU.mult)

                    def new_pt(q5, i):
                        return ptpool.tile([128, 2, 1024], i8, tag="pt",
                                           name=f"pt_{j}_{q5}_{i}")

                    pts = {}
                    pending = []
                    for idx in range(PIPE_AHEAD):
                        q5, t = seq[idx]
                        if t % 2 == 0:
                            pts[(q5, t // 2)] = new_pt(q5, t // 2)
                        emit_exp(q5, t, st_exp(q5, t), pts[(q5, t // 2)])
                    for it, (q5, t) in enumerate(seq):
                        if t == 0:
                            ots[q5] = otps.tile([VW, 1024], f32, tag="ot",
                                                name=f"ot_{j}_{q5}")
                        # exp pipeline first so PE/engine queues never sit
                        # behind a blocked PV or a waiting recip
                        if it + PIPE_AHEAD < len(seq):
                            q5n, tn = seq[it + PIPE_AHEAD]
                            if tn % 2 == 0:
                                pts[(q5n, tn // 2)] = new_pt(q5n, tn // 2)
                            emit_exp(q5n, tn, st_exp(q5n, tn),
                                     pts[(q5n, tn // 2)])
                        if t % 2 == 1:
                            emit_pv(q5, t // 2, pts.pop((q5, t // 2)))
                        if t == TT - 1:
                            emit_evac_copy(q5)
                            pending.append((q5, t))
                        # deferred: the recip/bcast/mults of q5 run 4 tiles
                        # into q5+1 so the recip never heads the DVE queue
                        if pending and (t == 4 or it == len(seq) - 1):
                            emit_evac_norm(pending.pop(0)[0])
                        it_state["it"] = it
                        pull(filler, budget(it))
                    while pending:
                        emit_evac_norm(pending.pop(0)[0])

                import os
                phases = os.environ.get("KV2_PHASES", "s1,p0,p1,p2,proj").split(",")
                for j in range(CTG - 1):
                    if f"p{j}" in phases:
                        attention_pair(j)

                if "p2" in phases:
                    attention_pair(CTG - 1)

            # ------------------- projection tail phase -------------------
            with (
                tc.tile_pool(name="outsb", bufs=3) as outsb,
                tc.tile_pool(name="prps", bufs=2, space="PSUM") as prps,
            ):
                if True:
                    pwT_kpn = pwT.rearrange("(k p) n -> p k n", p=128)
                    pwf = persist.tile([128, CTG, C], f32r, tag="pwf")
                    nc.sync.dma_start(out=pwf[:], in_=pwT_kpn[:, :, :])

                    def proj_tok_tile(tt):
                        osb = outsb.tile([128, C], f16, tag="osb",
                                         name=f"osb_p{tt}")
                        for half in range(2):
                            ps = prps.tile([128, C // 2], f32, tag="pr",
                                           name=f"prps_{tt}_{half}")
                            for k in range(CTG):
                                nc.tensor.matmul(
                                    ps[:],
                                    attnT[k][:, tt * 128:(tt + 1) * 128],
                                    pwf[:, k, half * (C // 2):
                                        (half + 1) * (C // 2)],
                                    start=(k == 0), stop=(k == CTG - 1))
                                yield
                            dst = osb[:, half * (C // 2):(half + 1) * (C // 2)]
                            if half == 0:
                                nc.scalar.activation(dst, ps[:], AF.Copy)
                            else:
                                nc.vector.tensor_copy(dst, ps[:])
                        nc.sync.dma_start(
                            out=out[tt * 128:(tt + 1) * 128, :], in_=osb[:])

                    rest = range(TT) if "proj" in phases else []
                    for tt in rest:
                        for _ in proj_tok_tile(tt):
                            pass
                    if dbg:
                        nc.sync.dma_start(out=dbg["q8"][:, :],
                                          in_=q8[0].bitcast(mybir.dt.uint8)[:, :, :])
                        nc.sync.dma_start(out=dbg["k8"][:, :],
                                          in_=k8[0].bitcast(mybir.dt.uint8)[:, :, :])
                        nc.sync.dma_start(out=dbg["vh"][:, :],
                                          in_=vHi[0].bitcast(mybir.dt.uint8)[:, :, :, :])
                        nc.sync.dma_start(out=dbg["vl"][:, :],
                                          in_=vLo[0].bitcast(mybir.dt.uint8)[:, :, :, :])
                        nc.sync.dma_start(out=dbg["at"][:, :],
                                          in_=attnT[0].bitcast(f32)[:, :])


def _prepare_inputs(x, qkv_w, qkv_b, proj_w, proj_b):
    """Host-side shard preparation (numpy quantize/reshape/transpose)."""
    x = np.asarray(x, dtype=np.float32)
    x8 = x.astype(E4)
    xr = (x - x8.astype(np.float32)).astype(E4)
    in_maps = []
    for core in range(NCORES):
        b, s = core // G, core % G
        rq = slice(s * CG, (s + 1) * CG)
        rk = slice(C + s * CG, C + (s + 1) * CG)
        rv = slice(2 * C + s * CG, 2 * C + (s + 1) * CG)
        wqk = np.concatenate([qkv_w[rq], qkv_w[rk]], 0).T  # [768, 768]
        wqk8 = (WS * wqk).astype(E4)
        wqkr = (WS * wqk - wqk8.astype(np.float32)).astype(E4)
        wv = qkv_w[rv].T  # [768, 384]
        wv8 = (WS * wv).astype(E4)
        wvr = (WS * wv - wv8.astype(np.float32)).astype(E4)
        bvec = np.concatenate([qkv_b[rq], qkv_b[rk]])
        bmat = bvec.reshape(2 * CTG, 128).T
        pwT = np.ascontiguousarray(proj_w.T[s * CG:(s + 1) * CG, :]).astype(
            np.float32)
        in_maps.append({
            "x8T": np.ascontiguousarray(x8[b].T),
            "xrT": np.ascontiguousarray(xr[b].T),
            "wqk8": np.ascontiguousarray(wqk8),
            "wqkr": np.ascontiguousarray(wqkr),
            "wv8": np.ascontiguousarray(wv8),
            "wvr": np.ascontiguousarray(wvr),
            "bqkW": np.ascontiguousarray(WS * bmat).astype(np.float32),
            "bqkQ": np.ascontiguousarray(QS * bmat).astype(np.float32),
            "pwT": pwT,
        })
    return in_maps


def kernel(x, qkv_w, qkv_b, proj_w, proj_b):
    x = np.asarray(x, dtype=np.float32)
    qkv_w = np.asarray(qkv_w, dtype=np.float32)
    qkv_b = np.asarray(qkv_b, dtype=np.float32)
    proj_w = np.asarray(proj_w, dtype=np.float32)
    proj_b = np.asarray(proj_b, dtype=np.float32)

    if "nc" not in _CACHE:
        _CACHE["nc"] = build_nc(reps=1)
    nc = _CACHE["nc"]

    in_maps = _prepare_inputs(x, qkv_w, qkv_b, proj_w, proj_b)
    res = run_bass_kernel_spmd(nc, in_maps, list(range(NCORES)))

    # host-side bias fold: v-bias through proj + proj bias
    pb_total = (proj_b + qkv_b[2 * C:] @ proj_w.T).astype(np.float32)
    out = np.empty((B, N, C), dtype=np.float32)
    for b in range(B):
        out[b] = res.results[G * b]["out"].astype(np.float32)
        out[b] += res.results[G * b + 1]["out"].astype(np.float32)
        out[b] += pb_total
    return out
